# revision 1
# baseline (speedup 1.0000x reference)
"""Trainium2 Bass kernel for nn_Decoder_78305843741218.

2-layer GRU decoder, autoregressive over T=256 steps, batch 1024.
Sharding: data-parallel over batch -> 128 samples/core on 8 cores.

Design (per core, per step):
  - All GEMMs on TensorE with stationary = transposed activations
    (feature-major bf16 k-tiles), moving = W.T bf16 chunks (N<=512),
    accumulating f32 in PSUM.  out = h @ W.T lands batch-major.
  - glob @ W_ih0[:, :H].T is step-invariant: precomputed on host (fp64)
    into full-rank consts c0rz/c0n, added on VectorE.
  - Per-feature biases enter PSUM via rank-1 matmuls (ones x bias-row).
  - Cell math batch-major; ScalarE uses ONLY the sigmoid act table
    (tanh via 2*sigmoid(2x)-1, softmax exp via sigmoid(x)/sigmoid(-x))
    so no per-step act-table reloads.
  - Recurrent state h0/h1 kept fp32; cast to bf16 + DMA-transposed
    (xbar) each step into 8 separate k-tiles (fine-grained deps);
    h0 transposes ride the SP hwdge queue, h1 transposes the ACT hwdge
    queue (both queues carry transposes only -> no xbar-mode flips);
    x^T is built on the PE (2 transpose matmuls + copies).
  - W_hh1 streamed from HBM every step on the gpsimd swdge queue.
  - Redundant Ldweights (same stationary) are deduped post-Tile so
    same-weight matmuls stream back-to-back on the PE.
  - PSUM: tags r/z/hn/in of 2 banks each = exactly 8 banks; fc and the
    x-transposes reuse tags 'in'/'hn' (true-dependency-free reuse).
  - Steps run under tc.For_i with an unrolled body to bound compile
    time; the last unrolled step writes the recurrent tiles into fixed
    "state" tiles read by the next iteration.

Output is stored T-major (T*128, D) per core; host reassembles.
"""

import os
import numpy as np
import ml_dtypes

H = 1024
D = 256
T = 256
P = 128
NCORES = 8
KT = H // P          # 8 k-tiles for H-dim contraction
KX = D // P          # 2 k-tiles for x contraction

_prog_cache = {}


def _build_program(t_steps=T, unroll=8):
    import concourse.bass as bass
    import concourse.bacc as bacc
    import concourse.mybir as mybir
    import concourse.tile as tile
    from contextlib import ExitStack

    f32 = mybir.dt.float32
    bf16 = mybir.dt.bfloat16
    AF = mybir.ActivationFunctionType
    ALU = mybir.AluOpType

    full_unroll = unroll >= t_steps
    if not full_unroll:
        assert t_steps % unroll == 0
    n_iter = 1 if full_unroll else t_steps // unroll

    nc = bacc.Bacc(None, target_bir_lowering=False)

    # ---- I/O ----
    d_whh0t = nc.dram_tensor("whh0t", (KT, P, 3 * H), bf16, kind="ExternalInput")
    d_wih1t = nc.dram_tensor("wih1t", (KT, P, 3 * H), bf16, kind="ExternalInput")
    d_whh1t = nc.dram_tensor("whh1t", (KT, P, 3 * H), bf16, kind="ExternalInput")
    d_wih0xt = nc.dram_tensor("wih0xt", (KX, P, 3 * H), bf16, kind="ExternalInput")
    d_wfct = nc.dram_tensor("wfct", (KT, P, D), bf16, kind="ExternalInput")
    d_h0 = nc.dram_tensor("h0_init", (P, H), f32, kind="ExternalInput")
    d_h1 = nc.dram_tensor("h1_init", (P, H), f32, kind="ExternalInput")
    d_h0t = nc.dram_tensor("h0t_init", (P, H), bf16, kind="ExternalInput")
    d_h1t = nc.dram_tensor("h1t_init", (P, H), bf16, kind="ExternalInput")
    d_x0t = nc.dram_tensor("x0t_init", (P, D), bf16, kind="ExternalInput")
    d_c0rz = nc.dram_tensor("c0rz", (P, 2 * H), f32, kind="ExternalInput")
    d_c0n = nc.dram_tensor("c0n", (P, H), f32, kind="ExternalInput")
    d_bhh0n = nc.dram_tensor("bhh0n", (1, H), bf16, kind="ExternalInput")
    d_brz1 = nc.dram_tensor("brz1", (1, 2 * H), bf16, kind="ExternalInput")
    d_bih1n = nc.dram_tensor("bih1n", (1, H), bf16, kind="ExternalInput")
    d_bhh1n = nc.dram_tensor("bhh1n", (1, H), bf16, kind="ExternalInput")
    d_bfc = nc.dram_tensor("bfc", (1, D), bf16, kind="ExternalInput")
    d_ident = nc.dram_tensor("ident", (P, P), bf16, kind="ExternalInput")
    d_res = nc.dram_tensor("res", (t_steps * P, D), f32, kind="ExternalOutput")

    with tile.TileContext(nc) as tc, ExitStack() as ctx:
        const = ctx.enter_context(tc.tile_pool(name="const", bufs=1))
        act = ctx.enter_context(tc.tile_pool(name="act", bufs=2))
        w1p = ctx.enter_context(tc.tile_pool(name="w1p", bufs=3))
        ps = ctx.enter_context(tc.tile_pool(name="ps", bufs=1, space="PSUM"))

        # ---- static loads (gpsimd swdge; one-time) ----
        _gc = [0]

        def gload(shape, dtype, src, name=None):
            _gc[0] += 1
            t = const.tile(shape, dtype, name=name or f"cst{_gc[0]}")
            nc.scalar.dma_start(t, src)
            return t

        whh0t = gload([P, KT, 3 * H], bf16, d_whh0t.rearrange("k p n -> p k n"))
        wih1t = gload([P, KT, 3 * H], bf16, d_wih1t.rearrange("k p n -> p k n"))
        wih0xt = gload([P, KX, 3 * H], bf16, d_wih0xt.rearrange("k p n -> p k n"))
        wfct = gload([P, KT, D], bf16, d_wfct.rearrange("k p n -> p k n"))
        c0rz = gload([P, 2 * H], f32, d_c0rz[:])
        c0n = gload([P, H], f32, d_c0n[:])
        h0 = gload([P, H], f32, d_h0[:])
        h1 = gload([P, H], f32, d_h1[:])
        ident = gload([P, P], bf16, d_ident[:])
        bhh0n = gload([1, H], bf16, d_bhh0n[:])
        brz1 = gload([1, 2 * H], bf16, d_brz1[:])
        bih1n = gload([1, H], bf16, d_bih1n[:])
        bhh1n = gload([1, H], bf16, d_bhh1n[:])
        bfc = gload([1, D], bf16, d_bfc[:])
        ones = const.tile([1, P], bf16)
        nc.vector.memset(ones, 1.0)

        # recurrent transposed-state k-tiles (fixed addresses for the loop)
        h0t_st = []
        h1t_st = []
        for j in range(KT):
            h0t_st.append(gload([P, P], bf16, d_h0t[:, j * P:(j + 1) * P], name=f"h0t_st{j}"))
            h1t_st.append(gload([P, P], bf16, d_h1t[:, j * P:(j + 1) * P], name=f"h1t_st{j}"))
        xt_st = [gload([P, P], bf16, d_x0t[:, j * P:(j + 1) * P], name=f"xt_st{j}") for j in range(KX)]

        def rank1(ps_t, row, n, stop=True):
            for c in range((n + 511) // 512):
                w = min(512, n - c * 512)
                nc.tensor.matmul(ps_t[:, c * 512:c * 512 + w], ones,
                                 row[:, c * 512:c * 512 + w],
                                 start=False, stop=stop)

        def gemm_h(ps_r, ps_z, ps_hn, w_tiles, stat, start, stop):
            for k in range(KT):
                lhsT = stat[k]
                w_k = w_tiles(k)
                for c in range(2):
                    nc.tensor.matmul(ps_r[:, c * 512:(c + 1) * 512], lhsT,
                                     w_k[:, c * 512:(c + 1) * 512],
                                     start=start and k == 0, stop=False)
                for c in range(2):
                    nc.tensor.matmul(ps_z[:, c * 512:(c + 1) * 512], lhsT,
                                     w_k[:, H + c * 512:H + (c + 1) * 512],
                                     start=start and k == 0, stop=False)
                for c in range(2):
                    nc.tensor.matmul(ps_hn[:, c * 512:(c + 1) * 512], lhsT,
                                     w_k[:, 2 * H + c * 512:2 * H + (c + 1) * 512],
                                     start=start and k == 0,
                                     stop=stop and k == KT - 1)

        state = {"h0t": h0t_st, "h1t": h1t_st, "xt": xt_st}

        def emit_step(res_row0, last_of_body):
            cur_h0t, cur_h1t, cur_xt = state["h0t"], state["h1t"], state["xt"]

            # ---- W_hh1 stream (gpsimd swdge, prefetch) ----
            w1land = []
            for k in range(KT):
                w1k = w1p.tile([P, 3 * H], bf16, tag="w1")
                nc.scalar.dma_start(w1k, d_whh1t[k])
                w1land.append(w1k)

            # ---- Layer 0 GEMMs ----
            ps_r = ps.tile([P, H], f32, tag="ps_r")
            ps_z = ps.tile([P, H], f32, tag="ps_z")
            ps_hn = ps.tile([P, H], f32, tag="ps_hn")
            ps_in = ps.tile([P, H], f32, tag="ps_in")

            gemm_h(ps_r, ps_z, ps_hn, lambda k: whh0t[:, k, :], cur_h0t,
                   start=True, stop=False)
            for k in range(KX):
                lhsT = cur_xt[k]
                w_k = wih0xt[:, k, :]
                for c in range(2):
                    nc.tensor.matmul(ps_r[:, c * 512:(c + 1) * 512], lhsT,
                                     w_k[:, c * 512:(c + 1) * 512],
                                     start=False, stop=k == KX - 1)
                for c in range(2):
                    nc.tensor.matmul(ps_z[:, c * 512:(c + 1) * 512], lhsT,
                                     w_k[:, H + c * 512:H + (c + 1) * 512],
                                     start=False, stop=k == KX - 1)
                for c in range(2):
                    nc.tensor.matmul(ps_in[:, c * 512:(c + 1) * 512], lhsT,
                                     w_k[:, 2 * H + c * 512:2 * H + (c + 1) * 512],
                                     start=k == 0, stop=k == KX - 1)
            rank1(ps_hn, bhh0n, H, stop=True)

            # ---- Cell 0 (sigmoid-table only) ----
            rz = act.tile([P, 2 * H], f32, tag="rz", bufs=1)
            g1 = act.tile([P, H], f32, tag="g1", bufs=1)
            g2 = act.tile([P, H], f32, tag="g2", bufs=1)
            n_sb = act.tile([P, H], f32, tag="n_sb", bufs=2)
            nc.vector.tensor_add(ps_r, ps_r, c0rz[:, :H])
            nc.scalar.activation(rz[:, :H], ps_r, AF.Sigmoid)
            nc.vector.tensor_add(ps_z, ps_z, c0rz[:, H:])
            nc.scalar.activation(rz[:, H:], ps_z, AF.Sigmoid)
            nc.vector.tensor_mul(g1, rz[:, :H], ps_hn)          # r * ghn
            nc.vector.tensor_add(g2, ps_in, c0n)                # gin + c0n
            nc.vector.tensor_add(n_sb, g1, g2)
            # tanh(x) = 2*sigmoid(2x) - 1
            nc.scalar.activation(n_sb, n_sb, AF.Sigmoid, scale=2.0)
            nc.vector.tensor_scalar(n_sb, n_sb, 2.0, 1.0, ALU.mult, ALU.subtract)
            g1b = act.tile([P, H], f32, tag="g1", bufs=1)
            nc.vector.tensor_sub(g1b, h0, n_sb)                 # h - n
            g2b = act.tile([P, H], f32, tag="g2", bufs=1)
            nc.vector.tensor_mul(g2b, rz[:, H:], g1b)           # z*(h-n)
            nc.vector.tensor_add(h0, g2b, n_sb)                 # h' = n + z*(h-n)
            h0bf = act.tile([P, H], bf16, tag="hbf", bufs=2)
            nc.scalar.copy(h0bf, h0)
            if last_of_body:
                h0t = h0t_st
            else:
                h0t = [act.tile([P, P], bf16, tag=f"h0t{j}", bufs=2, name=f"h0t_{j}")
                       for j in range(KT)]
            for j in range(KT):
                nc.sync.dma_start_transpose(h0t[j], h0bf[:, j * P:(j + 1) * P])

            # ---- Layer 1 GEMMs ----
            ps_r1 = ps.tile([P, H], f32, tag="ps_r")
            ps_z1 = ps.tile([P, H], f32, tag="ps_z")
            ps_hn1 = ps.tile([P, H], f32, tag="ps_hn")
            ps_in1 = ps.tile([P, H], f32, tag="ps_in")

            gemm_h(ps_r1, ps_z1, ps_hn1, lambda k: w1land[k], cur_h1t,
                   start=True, stop=False)
            rank1(ps_hn1, bhh1n, H, stop=True)
            for k in range(KT):
                lhsT = h0t[k]
                w_k = wih1t[:, k, :]
                for c in range(2):
                    nc.tensor.matmul(ps_r1[:, c * 512:(c + 1) * 512], lhsT,
                                     w_k[:, c * 512:(c + 1) * 512],
                                     start=False, stop=False)
                for c in range(2):
                    nc.tensor.matmul(ps_z1[:, c * 512:(c + 1) * 512], lhsT,
                                     w_k[:, H + c * 512:H + (c + 1) * 512],
                                     start=False, stop=False)
                for c in range(2):
                    nc.tensor.matmul(ps_in1[:, c * 512:(c + 1) * 512], lhsT,
                                     w_k[:, 2 * H + c * 512:2 * H + (c + 1) * 512],
                                     start=k == 0, stop=False)
            rank1(ps_r1, brz1[:, :H], H, stop=True)
            rank1(ps_z1, brz1[:, H:], H, stop=True)
            rank1(ps_in1, bih1n, H, stop=True)

            # ---- Cell 1 ----
            rz1 = act.tile([P, 2 * H], f32, tag="rz", bufs=1)
            n1_sb = act.tile([P, H], f32, tag="n_sb", bufs=2)
            nc.scalar.activation(rz1[:, :H], ps_r1, AF.Sigmoid)
            nc.scalar.activation(rz1[:, H:], ps_z1, AF.Sigmoid)
            g1c = act.tile([P, H], f32, tag="g1", bufs=1)
            nc.vector.tensor_mul(g1c, rz1[:, :H], ps_hn1)
            nc.vector.tensor_add(n1_sb, g1c, ps_in1)
            nc.scalar.activation(n1_sb, n1_sb, AF.Sigmoid, scale=2.0)
            nc.vector.tensor_scalar(n1_sb, n1_sb, 2.0, 1.0, ALU.mult, ALU.subtract)
            g1d = act.tile([P, H], f32, tag="g1", bufs=1)
            nc.vector.tensor_sub(g1d, h1, n1_sb)
            g2c = act.tile([P, H], f32, tag="g2", bufs=1)
            nc.vector.tensor_mul(g2c, rz1[:, H:], g1d)
            nc.vector.tensor_add(h1, g2c, n1_sb)
            h1bf = act.tile([P, H], bf16, tag="hbf", bufs=2)
            nc.scalar.copy(h1bf, h1)
            if last_of_body:
                h1t = h1t_st
            else:
                h1t = [act.tile([P, P], bf16, tag=f"h1t{j}", bufs=2, name=f"h1t_{j}")
                       for j in range(KT)]
            for j in range(KT):
                nc.sync.dma_start_transpose(h1t[j], h1bf[:, j * P:(j + 1) * P])

            # ---- FC + activations ----
            ps_fc = ps.tile([P, D], f32, tag="ps_in")
            for k in range(KT):
                nc.tensor.matmul(ps_fc, h1t[k], wfct[:, k, :],
                                 start=k == 0, stop=False)
            rank1(ps_fc, bfc, D, stop=True)

            xf = act.tile([P, D], f32, tag="xf", bufs=2)
            sigp = act.tile([P, 47], f32, tag="sigp", bufs=2)
            sign = act.tile([P, 47], f32, tag="sign", bufs=2)
            s12 = act.tile([P, 2], f32, tag="s12", bufs=2)
            r12 = act.tile([P, 2], f32, tag="r12", bufs=2)
            # exp(x) = sigmoid(x) / sigmoid(-x); softmax is scale-invariant
            nc.scalar.activation(xf[:, 47:D], ps_fc[:, 47:D], AF.Sigmoid)
            nc.scalar.activation(sigp, ps_fc[:, 0:47], AF.Sigmoid)
            nc.scalar.activation(sign, ps_fc[:, 0:47], AF.Sigmoid, scale=-1.0)
            nc.vector.reciprocal(sign, sign)
            nc.vector.scalar_tensor_tensor(
                xf[:, 0:32], sigp[:, 0:32], 1.0, sign[:, 0:32],
                ALU.mult, ALU.mult, accum_out=s12[:, 0:1])
            nc.vector.scalar_tensor_tensor(
                xf[:, 32:47], sigp[:, 32:47], 1.0, sign[:, 32:47],
                ALU.mult, ALU.mult, accum_out=s12[:, 1:2])
            nc.vector.reciprocal(r12, s12)
            nc.vector.tensor_scalar_mul(xf[:, 0:32], xf[:, 0:32], r12[:, 0:1])
            nc.vector.tensor_scalar_mul(xf[:, 32:47], xf[:, 32:47], r12[:, 1:2])
            xbf = act.tile([P, D], bf16, tag="xbf", bufs=2)
            nc.vector.tensor_copy(xbf, xf)
            # x^T on the PE (2 transpose matmuls into one psum bank + copies)
            ps_xt = ps.tile([P, KX, P], bf16, tag="ps_hn")
            if last_of_body:
                xt = xt_st
            else:
                xt = [act.tile([P, P], bf16, tag=f"xt{j}", bufs=2, name=f"xt_{j}")
                      for j in range(KX)]
            for j in range(KX):
                nc.tensor.transpose(ps_xt[:, j, :], xbf[:, j * P:(j + 1) * P],
                                    ident)
                nc.scalar.copy(xt[j], ps_xt[:, j, :])
            nc.gpsimd.dma_start(d_res[bass.ds(res_row0, P), :], xf)

            state["h0t"], state["h1t"], state["xt"] = h0t, h1t, xt

        if full_unroll:
            for t in range(t_steps):
                emit_step(t * P, last_of_body=False)
        else:
            et = mybir.EngineType
            with tc.For_i(0, n_iter, 1,
                          hint_engines=(et.PE, et.DVE, et.Activation,
                                        et.SP, et.Pool)) as iv:
                row_base = iv * (unroll * P)
                for j in range(unroll):
                    emit_step(row_base + j * P, last_of_body=(j == unroll - 1))

    _dedupe_ldweights(nc, mybir)
    nc.finalize()
    return nc


def _dedupe_ldweights(nc, mybir):
    """Drop redundant back-to-back Ldweights of the same stationary tile.

    Tile lowers every matmul to Ldweights+Matmult; reloading identical
    weights forces each matmul to run isolated (pay the full array drain)
    instead of streaming back-to-back.  Within one block, an Ldweights
    whose source AP equals the previous one's and which carries no sync
    waits/updates is a no-op for the PE array state -> remove it.  (A
    rewritten weight tile always induces a wait via Tile's RAW deps, and
    pool tiles get fresh memrefs per allocation, so "same AP + no waits"
    is safe.  Tracker resets per block, so loop back-edges are safe.)
    """
    import orjson
    removed = 0
    for func in nc.m.functions:
        for blk in func.blocks:
            last_key = None
            kept = []
            blk_removed = 0
            for inst in blk.instructions:
                if getattr(inst, "engine", None) == mybir.EngineType.PE:
                    d = orjson.loads(mybir.instruction_to_pretty_json_string(inst))
                    op = d.get("opcode")
                    if op == "Ldweights":
                        si = d.get("sync_info") or {}
                        key = orjson.dumps(
                            (d.get("ins"), d.get("tile_position"),
                             d.get("tile_size"), d.get("perf_mode"),
                             d.get("is_transpose")))
                        if (key == last_key and not si.get("on_wait")
                                and not si.get("on_update")):
                            removed += 1
                            blk_removed += 1
                            continue
                        last_key = key
                kept.append(inst)
            if blk_removed:
                blk.instructions[:] = kept
    return removed


def _host_prep(inputs):
    """Build per-core input maps."""
    bf = ml_dtypes.bfloat16
    embed = np.ascontiguousarray(np.asarray(inputs["embed"], dtype=np.float32))
    dynamics = np.asarray(inputs["dynamics"], dtype=np.float32)
    W_ih0 = np.asarray(inputs["W_ih0"], dtype=np.float32)
    W_hh0 = np.asarray(inputs["W_hh0"], dtype=np.float32)
    b_ih0 = np.asarray(inputs["b_ih0"], dtype=np.float32)
    b_hh0 = np.asarray(inputs["b_hh0"], dtype=np.float32)
    W_ih1 = np.asarray(inputs["W_ih1"], dtype=np.float32)
    W_hh1 = np.asarray(inputs["W_hh1"], dtype=np.float32)
    b_ih1 = np.asarray(inputs["b_ih1"], dtype=np.float32)
    b_hh1 = np.asarray(inputs["b_hh1"], dtype=np.float32)
    W_fc = np.asarray(inputs["W_fc"], dtype=np.float32)
    b_fc = np.asarray(inputs["b_fc"], dtype=np.float32)

    glob = embed[:, :H]
    h0i = embed[:, H:2 * H]
    h1i = embed[:, 2 * H:3 * H]
    x0 = dynamics[:, 0, :]

    c0 = (glob.astype(np.float64) @ W_ih0[:, :H].T.astype(np.float64)).astype(np.float32)
    c0 += b_ih0
    c0rz = np.ascontiguousarray(c0[:, :2 * H] + b_hh0[:2 * H])
    c0n = np.ascontiguousarray(c0[:, 2 * H:])

    def ktiles(wT, kt):
        return np.ascontiguousarray(wT.reshape(kt, P, wT.shape[1]).astype(bf))

    shared = {
        "whh0t": ktiles(W_hh0.T, KT),
        "wih1t": ktiles(W_ih1.T, KT),
        "whh1t": ktiles(W_hh1.T, KT),
        "wih0xt": ktiles(np.ascontiguousarray(W_ih0[:, H:].T), KX),
        "wfct": ktiles(W_fc.T, KT),
        "bhh0n": b_hh0[2 * H:].reshape(1, H).astype(bf),
        "brz1": (b_ih1 + b_hh1)[:2 * H].reshape(1, 2 * H).astype(bf),
        "bih1n": b_ih1[2 * H:].reshape(1, H).astype(bf),
        "bhh1n": b_hh1[2 * H:].reshape(1, H).astype(bf),
        "bfc": b_fc.reshape(1, D).astype(bf),
        "ident": np.eye(P, dtype=np.float32).astype(bf),
    }

    def blockT(a):
        out = np.empty_like(a)
        for j in range(a.shape[1] // P):
            out[:, j * P:(j + 1) * P] = a[:, j * P:(j + 1) * P].T
        return out

    in_maps = []
    for c in range(NCORES):
        s = slice(c * P, (c + 1) * P)
        m = dict(shared)
        m["h0_init"] = np.ascontiguousarray(h0i[s])
        m["h1_init"] = np.ascontiguousarray(h1i[s])
        m["h0t_init"] = blockT(h0i[s]).astype(bf)
        m["h1t_init"] = blockT(h1i[s]).astype(bf)
        m["x0t_init"] = blockT(x0[s]).astype(bf)
        m["c0rz"] = c0rz[s].copy()
        m["c0n"] = c0n[s].copy()
        in_maps.append(m)
    return in_maps


def _install_neff_cache():
    """Cache walrus-compiled NEFFs keyed by BIR hash (compile is minutes)."""
    import hashlib
    import shutil
    import concourse.bass_utils as bu
    import concourse.bass2jax as b2j

    if getattr(bu, "_decoder_neff_cache", False):
        return
    orig = bu.compile_bir_kernel

    def cached(bir_json, tmpdir, neff_name="file.neff"):
        try:
            h = hashlib.sha256(bir_json).hexdigest()[:32]
            cdir = os.path.join(os.path.expanduser("~"), ".cache", "bass_neff")
            os.makedirs(cdir, exist_ok=True)
            cpath = os.path.join(cdir, h + ".neff")
            if os.path.exists(cpath):
                dst = os.path.join(tmpdir, "sg00")
                os.makedirs(dst, exist_ok=True)
                out = os.path.join(dst, neff_name)
                shutil.copy(cpath, out)
                return out
            out = orig(bir_json, tmpdir, neff_name)
            shutil.copy(out, cpath)
            return out
        except Exception:
            return orig(bir_json, tmpdir, neff_name)

    bu.compile_bir_kernel = cached
    b2j.compile_bir_kernel = cached
    bu._decoder_neff_cache = True


def kernel(**inputs):
    from concourse.bass_utils import run_bass_kernel_spmd

    _install_neff_cache()
    key = (T, 8)
    if key not in _prog_cache:
        _prog_cache[key] = _build_program(T, unroll=8)
    nc = _prog_cache[key]

    in_maps = _host_prep(inputs)
    out = run_bass_kernel_spmd(nc, in_maps, core_ids=list(range(NCORES)))
    res = np.concatenate(
        [r["res"].reshape(T, P, D).transpose(1, 0, 2) for r in out.results],
        axis=0)
    return np.ascontiguousarray(res, dtype=np.float32)



# revision 5
# speedup vs baseline: 1.4790x; 1.4790x over previous
"""Trainium2 Bass kernel for nn_Decoder_78305843741218.

2-layer GRU decoder, autoregressive over T=256 steps, batch 1024.
Sharding: data-parallel over batch -> 128 samples/core on 8 cores.

v2 design (per core, per step):
  - All big GEMMs in fp8-e4m3 DoubleRow (2 contraction rows/cycle):
    stationary = transposed activations as fp8 pair-tiles [128,2,128]
    (scaled x16), moving = W^T fp8 pair-chunks [128,2,512] (scaled x64).
    PSUM accumulates at 1024x true scale; descaled for free via the
    activation-function scale argument (sigmoid/tanh scale=1/1024).
  - All weights SBUF-resident in fp8 (~10.5 MB) - no HBM streaming.
  - Per-feature constants (glob @ W_ih0[:,:H].T + biases, all x1024,
    bf16) enter PSUM via identity-stationary matmuls.
  - Gate banks are processed in 512-wide halves: 8 PSUM banks =
    {r,z,hn,in} x {lo,hi}; cell math for half v runs on DVE/ACT while
    the PE streams the other half / next layer, so the PE never idles
    long enough for HAM to re-throttle.
  - Cell math fp32 from PSUM; real Tanh (same ACT table set as
    Sigmoid); softmax exp via sigmoid(x)/sigmoid(-x).
  - Recurrent state h0/h1 fp32 in SBUF; per step: ACT-cast to bf16,
    DMA-transposed (xbar) to feature-major, ACT-cast (x16) to fp8
    pair-tiles.  h0 transposes ride the SP hwdge queue, h1+x rides
    the ACT hwdge queue (transposes only per queue -> no xbar flips).
  - Redundant Ldweights deduped post-Tile.
  - Steps run under tc.For_i with an unrolled body; the last unrolled
    step writes recurrent tiles into fixed "state" tiles.

Output is stored T-major (T*128, D) per core; host reassembles.
"""

import os
import numpy as np
import ml_dtypes

H = 1024
D = 256
T = 256
P = 128
NCORES = 8
KP = 4            # fp8 pair k-tiles for H-dim contraction
SW = 64.0         # weight scale in fp8
SH = 16.0         # activation scale in fp8
SC = SW * SH      # psum scale

_prog_cache = {}


def _build_program(t_steps=T, unroll=8):
    import concourse.bass as bass
    import concourse.bacc as bacc
    import concourse.mybir as mybir
    import concourse.tile as tile
    from contextlib import ExitStack

    f32 = mybir.dt.float32
    bf16 = mybir.dt.bfloat16
    f8 = mybir.dt.float8e4
    AF = mybir.ActivationFunctionType
    DR = mybir.MatmulPerfMode.DoubleRow

    full_unroll = unroll >= t_steps
    if not full_unroll:
        assert t_steps % unroll == 0
    n_iter = 1 if full_unroll else t_steps // unroll

    nc = bacc.Bacc(None, target_bir_lowering=False)

    # ---- I/O ----
    d_w8hh0 = nc.dram_tensor("w8hh0", (KP, P, 2, 3 * H), f8, kind="ExternalInput")
    d_w8ih1 = nc.dram_tensor("w8ih1", (KP, P, 2, 3 * H), f8, kind="ExternalInput")
    d_w8hh1 = nc.dram_tensor("w8hh1", (KP, P, 2, 3 * H), f8, kind="ExternalInput")
    d_w8x = nc.dram_tensor("w8x", (1, P, 2, 3 * H), f8, kind="ExternalInput")
    d_w8fc = nc.dram_tensor("w8fc", (KP, P, 2, D), f8, kind="ExternalInput")
    d_const0 = nc.dram_tensor("const0", (P, 4 * H), bf16, kind="ExternalInput")
    d_const1 = nc.dram_tensor("const1", (P, 4 * H), bf16, kind="ExternalInput")
    d_constfc = nc.dram_tensor("constfc", (P, D), bf16, kind="ExternalInput")
    d_ident = nc.dram_tensor("ident", (P, P), bf16, kind="ExternalInput")
    d_h0 = nc.dram_tensor("h0_init", (P, H), f32, kind="ExternalInput")
    d_h1 = nc.dram_tensor("h1_init", (P, H), f32, kind="ExternalInput")
    d_h0t8 = nc.dram_tensor("h0t8_init", (P, 8, P), f8, kind="ExternalInput")
    d_h1t8 = nc.dram_tensor("h1t8_init", (P, 8, P), f8, kind="ExternalInput")
    d_xt8 = nc.dram_tensor("xt8_init", (P, 2, P), f8, kind="ExternalInput")
    d_res = nc.dram_tensor("res", (t_steps * P, D), f32, kind="ExternalOutput")

    with tile.TileContext(nc) as tc, ExitStack() as ctx:
        const = ctx.enter_context(tc.tile_pool(name="const", bufs=1))
        act = ctx.enter_context(tc.tile_pool(name="act", bufs=2))
        ps = ctx.enter_context(tc.tile_pool(name="ps", bufs=1, space="PSUM"))

        _gc = [0]

        def gload(shape, dtype, src, name=None):
            _gc[0] += 1
            t = const.tile(shape, dtype, name=name or f"cst{_gc[0]}")
            nc.gpsimd.dma_start(t, src)
            return t

        w8hh0 = gload([P, KP, 2, 3 * H], f8, d_w8hh0.rearrange("k p j n -> p k j n"))
        w8ih1 = gload([P, KP, 2, 3 * H], f8, d_w8ih1.rearrange("k p j n -> p k j n"))
        w8hh1 = gload([P, KP, 2, 3 * H], f8, d_w8hh1.rearrange("k p j n -> p k j n"))
        w8x = gload([P, 1, 2, 3 * H], f8, d_w8x.rearrange("k p j n -> p k j n"))
        w8fc = gload([P, KP, 2, D], f8, d_w8fc.rearrange("k p j n -> p k j n"))
        const0 = gload([P, 4 * H], bf16, d_const0[:])
        const1 = gload([P, 4 * H], bf16, d_const1[:])
        constfc = gload([P, D], bf16, d_constfc[:])
        ident = gload([P, P], bf16, d_ident[:])
        h0 = gload([P, H], f32, d_h0[:])
        h1 = gload([P, H], f32, d_h1[:])
        # fixed-address recurrent fp8 stationary tiles (split lo/hi for
        # fine-grained deps: pairs 0-1 in a, 2-3 in b)
        h0t8a_st = gload([P, 4, P], f8, d_h0t8[:, 0:4, :], name="h0t8a_st")
        h0t8b_st = gload([P, 4, P], f8, d_h0t8[:, 4:8, :], name="h0t8b_st")
        h1t8a_st = gload([P, 4, P], f8, d_h1t8[:, 0:4, :], name="h1t8a_st")
        h1t8b_st = gload([P, 4, P], f8, d_h1t8[:, 4:8, :], name="h1t8b_st")
        xt8_st = gload([P, 2, P], f8, d_xt8[:], name="xt8_st")

        GATES = ("r", "z", "hn", "in")
        GOFF = {"r": 0, "z": 1, "hn": 2, "in": 3}

        state = {"h0t8": (h0t8a_st, h0t8b_st),
                 "h1t8": (h1t8a_st, h1t8b_st),
                 "xt8": xt8_st}

        def pair_ap(t8pair, kt):
            a, b = t8pair
            return (a if kt < 2 else b)[:, 2 * (kt % 2):2 * (kt % 2) + 2, :]

        def gemm_layer(lid, hh_t8, w_hh, ih_t8, n_ih_pairs, w_ih, cst):
            """Emit one GRU layer's gemms.  Returns psum tiles
            {(gate, v): tile}.  Banks: I-const, then hh pairs (r,z,hn),
            then ih pairs (r,z,in)."""
            psg = {}
            for v in (0, 1):
                for g in GATES:
                    pt = ps.tile([P, 512], f32, tag=f"{g}{v}")
                    psg[(g, v)] = pt
                    nc.tensor.matmul(
                        pt, ident,
                        cst[:, GOFF[g] * H + v * 512:GOFF[g] * H + (v + 1) * 512],
                        start=True, stop=False)
                for k in range(KP):
                    lhsT = pair_ap(hh_t8, k)
                    for g, nb in (("r", 0), ("z", H), ("hn", 2 * H)):
                        nc.tensor.matmul(
                            psg[(g, v)], lhsT,
                            w_hh[:, k, :, nb + v * 512:nb + (v + 1) * 512],
                            start=False, stop=(g == "hn" and k == KP - 1),
                            perf_mode=DR)
                for k in range(n_ih_pairs):
                    if n_ih_pairs == 1:
                        lhsT = ih_t8[:, 0:2, :]
                    else:
                        lhsT = pair_ap(ih_t8, k)
                    for g, nb in (("r", 0), ("z", H), ("in", 2 * H)):
                        nc.tensor.matmul(
                            psg[(g, v)], lhsT,
                            w_ih[:, k, :, nb + v * 512:nb + (v + 1) * 512],
                            start=False, stop=k == n_ih_pairs - 1,
                            perf_mode=DR)
            return psg

        def cell(lid, psg, h, t8_out, last_of_body, transpose_engine):
            """GRU cell from psum gates; updates h (f32, in place) and
            writes fp8 transposed pairs into t8_out (a, b)."""
            def ctile(shape, dt, tg, bufs=1):
                return act.tile(shape, dt, tag=tg, bufs=bufs, name=f"cl_{tg}")
            r_sb = [ctile([P, 512], f32, f"r{lid}{v}") for v in (0, 1)]
            z_sb = [ctile([P, 512], f32, f"z{lid}{v}") for v in (0, 1)]
            g1 = [ctile([P, 512], f32, f"a{lid}{v}") for v in (0, 1)]
            npre = [ctile([P, 512], f32, f"b{lid}{v}") for v in (0, 1)]
            n_t = [ctile([P, 512], f32, f"n{lid}{v}") for v in (0, 1)]
            t1 = [ctile([P, 512], f32, f"a{lid}{v}") for v in (0, 1)]
            t2 = [ctile([P, 512], f32, f"b{lid}{v}") for v in (0, 1)]
            hbf = act.tile([P, H], bf16, tag=f"hbf{lid}", bufs=2,
                           name=f"hbf{lid}")
            htbf = [ctile([P, 4, P], bf16, f"htbf{lid}{v}", bufs=2)
                    for v in (0, 1)]
            for v in (0, 1):
                nc.scalar.activation(r_sb[v], psg[("r", v)], AF.Sigmoid,
                                     scale=1.0 / SC)
                nc.scalar.activation(z_sb[v], psg[("z", v)], AF.Sigmoid,
                                     scale=1.0 / SC)
            for v in (0, 1):
                nc.vector.tensor_mul(g1[v], r_sb[v], psg[("hn", v)])
                nc.vector.tensor_add(npre[v], g1[v], psg[("in", v)])
                nc.scalar.activation(n_t[v], npre[v], AF.Tanh, scale=1.0 / SC)
            for v in (0, 1):
                hs = h[:, v * 512:(v + 1) * 512]
                nc.vector.tensor_sub(t1[v], hs, n_t[v])
                nc.vector.tensor_mul(t2[v], z_sb[v], t1[v])
                nc.vector.tensor_add(hs, t2[v], n_t[v])
                nc.scalar.copy(hbf[:, v * 512:(v + 1) * 512], hs)
                for j in range(4):
                    transpose_engine.dma_start_transpose(
                        htbf[v][:, j, :],
                        hbf[:, v * 512 + j * P:v * 512 + (j + 1) * P])
            if last_of_body:
                out = t8_out
            else:
                out = (act.tile([P, 4, P], f8, tag=f"t8_{lid}a", bufs=2,
                                name=f"t8_{lid}a"),
                       act.tile([P, 4, P], f8, tag=f"t8_{lid}b", bufs=2,
                                name=f"t8_{lid}b"))
            for v in (0, 1):
                nc.scalar.mul(out[v], htbf[v], SH)
            return out

        def emit_step(res_row0, last_of_body):
            cur_h0t8 = state["h0t8"]
            cur_h1t8 = state["h1t8"]
            cur_xt8 = state["xt8"]

            # ---- Layer 0 ----
            psg0 = gemm_layer(0, cur_h0t8, w8hh0, cur_xt8, 1, w8x, const0)
            new_h0t8 = cell(0, psg0, h0, (h0t8a_st, h0t8b_st) if last_of_body
                            else None, last_of_body, nc.sync)

            # ---- Layer 1 ----
            psg1 = gemm_layer(1, cur_h1t8, w8hh1, new_h0t8, KP, w8ih1, const1)
            new_h1t8 = cell(1, psg1, h1, (h1t8a_st, h1t8b_st) if last_of_body
                            else None, last_of_body, nc.scalar)

            # ---- FC ----
            ps_fc = ps.tile([P, 512], f32, tag="r0")
            nc.tensor.matmul(ps_fc[:, :D], ident, constfc, start=True,
                             stop=False)
            for k in range(KP):
                nc.tensor.matmul(ps_fc[:, :D], pair_ap(new_h1t8, k),
                                 w8fc[:, k, :, :], start=False,
                                 stop=k == KP - 1, perf_mode=DR)

            # ---- activations / softmax ----
            ALU = mybir.AluOpType
            xf = act.tile([P, D], f32, tag="xf", bufs=2)
            sigp = act.tile([P, 47], f32, tag="sigp", bufs=2)
            sign = act.tile([P, 47], f32, tag="sign", bufs=2)
            s12 = act.tile([P, 2], f32, tag="s12", bufs=2)
            r12 = act.tile([P, 2], f32, tag="r12", bufs=2)
            nc.scalar.activation(xf[:, 47:D], ps_fc[:, 47:D], AF.Sigmoid,
                                 scale=1.0 / SC)
            nc.scalar.activation(sigp, ps_fc[:, 0:47], AF.Sigmoid,
                                 scale=1.0 / SC)
            nc.scalar.activation(sign, ps_fc[:, 0:47], AF.Sigmoid,
                                 scale=-1.0 / SC)
            nc.vector.reciprocal(sign, sign)
            nc.vector.scalar_tensor_tensor(
                xf[:, 0:32], sigp[:, 0:32], 1.0, sign[:, 0:32],
                ALU.mult, ALU.mult, accum_out=s12[:, 0:1])
            nc.vector.scalar_tensor_tensor(
                xf[:, 32:47], sigp[:, 32:47], 1.0, sign[:, 32:47],
                ALU.mult, ALU.mult, accum_out=s12[:, 1:2])
            nc.vector.reciprocal(r12, s12)
            nc.vector.tensor_scalar_mul(xf[:, 0:32], xf[:, 0:32], r12[:, 0:1])
            nc.vector.tensor_scalar_mul(xf[:, 32:47], xf[:, 32:47], r12[:, 1:2])
            nc.gpsimd.dma_start(d_res[bass.ds(res_row0, P), :], xf)

            # ---- x -> fp8 pair tile for next step ----
            xbf = act.tile([P, D], bf16, tag="xbf", bufs=2)
            nc.vector.tensor_copy(xbf, xf)
            xtbf = act.tile([P, 2, P], bf16, tag="xtbf", bufs=2)
            for j in range(2):
                nc.scalar.dma_start_transpose(xtbf[:, j, :],
                                              xbf[:, j * P:(j + 1) * P])
            if last_of_body:
                xt8 = xt8_st
            else:
                xt8 = act.tile([P, 2, P], f8, tag="xt8", bufs=2)
            nc.scalar.mul(xt8, xtbf, SH)

            state["h0t8"], state["h1t8"], state["xt8"] = (
                new_h0t8, new_h1t8, xt8)

        if full_unroll:
            for t in range(t_steps):
                emit_step(t * P, last_of_body=False)
        else:
            et = mybir.EngineType
            with tc.For_i(0, n_iter, 1,
                          hint_engines=(et.PE, et.DVE, et.Activation,
                                        et.SP, et.Pool)) as iv:
                row_base = iv * (unroll * P)
                for j in range(unroll):
                    emit_step(row_base + j * P, last_of_body=(j == unroll - 1))

    _dedupe_ldweights(nc, mybir)
    nc.finalize()
    return nc


def _dedupe_ldweights(nc, mybir):
    """Drop redundant back-to-back Ldweights of the same stationary tile."""
    import orjson
    removed = 0
    for func in nc.m.functions:
        for blk in func.blocks:
            last_key = None
            kept = []
            blk_removed = 0
            for inst in blk.instructions:
                if getattr(inst, "engine", None) == mybir.EngineType.PE:
                    d = orjson.loads(mybir.instruction_to_pretty_json_string(inst))
                    op = d.get("opcode")
                    if op == "Ldweights":
                        si = d.get("sync_info") or {}
                        key = orjson.dumps(
                            (d.get("ins"), d.get("tile_position"),
                             d.get("tile_size"), d.get("perf_mode"),
                             d.get("is_transpose")))
                        if (key == last_key and not si.get("on_wait")
                                and not si.get("on_update")):
                            removed += 1
                            blk_removed += 1
                            continue
                        last_key = key
                kept.append(inst)
            if blk_removed:
                blk.instructions[:] = kept
    return removed


def _host_prep(inputs):
    """Build per-core input maps."""
    bf = ml_dtypes.bfloat16
    e4 = ml_dtypes.float8_e4m3
    embed = np.ascontiguousarray(np.asarray(inputs["embed"], dtype=np.float32))
    dynamics = np.asarray(inputs["dynamics"], dtype=np.float32)
    W_ih0 = np.asarray(inputs["W_ih0"], dtype=np.float32)
    W_hh0 = np.asarray(inputs["W_hh0"], dtype=np.float32)
    b_ih0 = np.asarray(inputs["b_ih0"], dtype=np.float32)
    b_hh0 = np.asarray(inputs["b_hh0"], dtype=np.float32)
    W_ih1 = np.asarray(inputs["W_ih1"], dtype=np.float32)
    W_hh1 = np.asarray(inputs["W_hh1"], dtype=np.float32)
    b_ih1 = np.asarray(inputs["b_ih1"], dtype=np.float32)
    b_hh1 = np.asarray(inputs["b_hh1"], dtype=np.float32)
    W_fc = np.asarray(inputs["W_fc"], dtype=np.float32)
    b_fc = np.asarray(inputs["b_fc"], dtype=np.float32)

    glob = embed[:, :H]
    h0i = embed[:, H:2 * H]
    h1i = embed[:, 2 * H:3 * H]
    x0 = dynamics[:, 0, :]

    c0 = (glob.astype(np.float64) @ W_ih0[:, :H].T.astype(np.float64)).astype(np.float32)
    c0 += b_ih0

    def pairize(wT, kp):
        # wT [K, N] -> [kp, P, 2, N] fp8, x SW
        K, N = wT.shape
        assert K == kp * 2 * P
        w8 = np.asarray(wT * SW, dtype=e4)
        return np.ascontiguousarray(w8.reshape(kp, 2, P, N).transpose(0, 2, 1, 3))

    def bcast(row):
        return np.broadcast_to(row, (P, row.shape[0]))

    const1 = np.concatenate([
        bcast((b_ih1 + b_hh1)[:H]),
        bcast((b_ih1 + b_hh1)[H:2 * H]),
        bcast(b_hh1[2 * H:]),
        bcast(b_ih1[2 * H:]),
    ], axis=1) * SC

    shared = {
        "w8hh0": pairize(np.ascontiguousarray(W_hh0.T), KP),
        "w8ih1": pairize(np.ascontiguousarray(W_ih1.T), KP),
        "w8hh1": pairize(np.ascontiguousarray(W_hh1.T), KP),
        "w8x": pairize(np.ascontiguousarray(W_ih0[:, H:].T), 1),
        "w8fc": pairize(np.ascontiguousarray(W_fc.T), KP),
        "const1": np.ascontiguousarray(const1).astype(bf),
        "constfc": np.ascontiguousarray(bcast(b_fc) * SC).astype(bf),
        "ident": np.eye(P, dtype=np.float32).astype(bf),
    }

    def t8(hslice, nslots):
        # [P(batch), nslots*P(features)] -> [P(part=feat within slot), slot, P(batch)]
        hT = np.ascontiguousarray(hslice.T)  # [F, P]
        return np.ascontiguousarray(
            np.asarray(hT * SH, dtype=e4).reshape(nslots, P, P).transpose(1, 0, 2))

    in_maps = []
    for c in range(NCORES):
        s = slice(c * P, (c + 1) * P)
        m = dict(shared)
        m["h0_init"] = np.ascontiguousarray(h0i[s])
        m["h1_init"] = np.ascontiguousarray(h1i[s])
        m["h0t8_init"] = t8(h0i[s], 8)
        m["h1t8_init"] = t8(h1i[s], 8)
        m["xt8_init"] = t8(x0[s], 2)
        const0 = np.concatenate([
            c0[s, :H] + b_hh0[:H],
            c0[s, H:2 * H] + b_hh0[H:2 * H],
            np.broadcast_to(b_hh0[2 * H:], (P, H)),
            c0[s, 2 * H:],
        ], axis=1) * SC
        m["const0"] = np.ascontiguousarray(const0).astype(bf)
        in_maps.append(m)
    return in_maps


def _install_neff_cache():
    """Cache walrus-compiled NEFFs keyed by BIR hash."""
    import hashlib
    import shutil
    import concourse.bass_utils as bu
    import concourse.bass2jax as b2j

    if getattr(bu, "_decoder_neff_cache", False):
        return
    orig = bu.compile_bir_kernel

    def cached(bir_json, tmpdir, neff_name="file.neff"):
        try:
            h = hashlib.sha256(bir_json).hexdigest()[:32]
            cdir = os.path.join(os.path.expanduser("~"), ".cache", "bass_neff")
            os.makedirs(cdir, exist_ok=True)
            cpath = os.path.join(cdir, h + ".neff")
            if os.path.exists(cpath):
                dst = os.path.join(tmpdir, "sg00")
                os.makedirs(dst, exist_ok=True)
                out = os.path.join(dst, neff_name)
                shutil.copy(cpath, out)
                return out
            out = orig(bir_json, tmpdir, neff_name)
            shutil.copy(out, cpath)
            return out
        except Exception:
            return orig(bir_json, tmpdir, neff_name)

    bu.compile_bir_kernel = cached
    b2j.compile_bir_kernel = cached
    bu._decoder_neff_cache = True


def _get_nc():
    key = ("v2", T, 8)
    if key not in _prog_cache:
        _prog_cache[key] = _build_program(T, unroll=8)
    return _prog_cache[key]


def kernel(**inputs):
    from concourse.bass_utils import run_bass_kernel_spmd

    _install_neff_cache()
    nc = _get_nc()
    in_maps = _host_prep(inputs)
    out = run_bass_kernel_spmd(nc, in_maps, core_ids=list(range(NCORES)))
    res = np.concatenate(
        [r["res"].reshape(T, P, D).transpose(1, 0, 2) for r in out.results],
        axis=0)
    return np.ascontiguousarray(res, dtype=np.float32)


# revision 8
# speedup vs baseline: 1.9256x; 1.3019x over previous
"""Trainium2 Bass kernel for nn_Decoder_78305843741218.

2-layer GRU decoder, autoregressive over T=256 steps, batch 1024.
Sharding: data-parallel over batch -> 128 samples/core on 8 cores.

v2 design (per core, per step):
  - All big GEMMs in fp8-e4m3 DoubleRow (2 contraction rows/cycle):
    stationary = transposed activations as fp8 pair-tiles [128,2,128]
    (scaled x16), moving = W^T fp8 pair-chunks [128,2,512] (scaled x64).
    PSUM accumulates at 1024x true scale; descaled for free via the
    activation-function scale argument (sigmoid/tanh scale=1/1024).
  - All weights SBUF-resident in fp8 (~10.5 MB) - no HBM streaming.
  - Per-feature constants (glob @ W_ih0[:,:H].T + biases, all x1024,
    bf16) enter PSUM via identity-stationary matmuls.
  - Gate banks are processed in 512-wide halves: 8 PSUM banks =
    {r,z,hn,in} x {lo,hi}; cell math for half v runs on DVE/ACT while
    the PE streams the other half / next layer, so the PE never idles
    long enough for HAM to re-throttle.
  - Cell math fp32 from PSUM; real Tanh (same ACT table set as
    Sigmoid); softmax exp via sigmoid(x)/sigmoid(-x).
  - Recurrent state h0/h1 fp32 in SBUF; per step: ACT-cast to bf16,
    DMA-transposed (xbar) to feature-major, ACT-cast (x16) to fp8
    pair-tiles.  h0 transposes ride the SP hwdge queue, h1+x rides
    the ACT hwdge queue (transposes only per queue -> no xbar flips).
  - Redundant Ldweights deduped post-Tile.
  - Steps run under tc.For_i with an unrolled body; the last unrolled
    step writes recurrent tiles into fixed "state" tiles.

Output is stored T-major (T*128, D) per core; host reassembles.
"""

import os
import numpy as np
import ml_dtypes

H = 1024
D = 256
T = 256
P = 128
NCORES = 8
KP = 4            # fp8 pair k-tiles for H-dim contraction
SW = 64.0         # weight scale in fp8
SH = 16.0         # activation scale in fp8
SC = SW * SH      # psum scale

_prog_cache = {}


def _build_program(t_steps=T, unroll=8):
    import concourse.bass as bass
    import concourse.bacc as bacc
    import concourse.mybir as mybir
    import concourse.tile as tile
    from contextlib import ExitStack

    f32 = mybir.dt.float32
    bf16 = mybir.dt.bfloat16
    f8 = mybir.dt.float8e4
    AF = mybir.ActivationFunctionType
    DR = mybir.MatmulPerfMode.DoubleRow

    full_unroll = unroll >= t_steps
    if not full_unroll:
        assert t_steps % unroll == 0
    n_iter = 1 if full_unroll else t_steps // unroll

    nc = bacc.Bacc(None, target_bir_lowering=False)

    # ---- I/O ----
    d_w8hh0 = nc.dram_tensor("w8hh0", (KP, P, 2, 3 * H), f8, kind="ExternalInput")
    d_w8ih1 = nc.dram_tensor("w8ih1", (KP, P, 2, 3 * H), f8, kind="ExternalInput")
    d_w8hh1 = nc.dram_tensor("w8hh1", (KP, P, 2, 3 * H), f8, kind="ExternalInput")
    d_w8x = nc.dram_tensor("w8x", (1, P, 2, 3 * H), f8, kind="ExternalInput")
    d_w8fc = nc.dram_tensor("w8fc", (KP, P, 2, D), f8, kind="ExternalInput")
    d_const0 = nc.dram_tensor("const0", (P, 4 * H), bf16, kind="ExternalInput")
    d_const1 = nc.dram_tensor("const1", (P, 4 * H), bf16, kind="ExternalInput")
    d_constfc = nc.dram_tensor("constfc", (P, D), bf16, kind="ExternalInput")
    d_ident = nc.dram_tensor("ident", (P, P), bf16, kind="ExternalInput")
    d_h0 = nc.dram_tensor("h0_init", (P, H), f32, kind="ExternalInput")
    d_h1 = nc.dram_tensor("h1_init", (P, H), f32, kind="ExternalInput")
    d_h0t8 = nc.dram_tensor("h0t8_init", (P, 8, P), f8, kind="ExternalInput")
    d_h1t8 = nc.dram_tensor("h1t8_init", (P, 8, P), f8, kind="ExternalInput")
    d_xt8 = nc.dram_tensor("xt8_init", (P, 2, P), f8, kind="ExternalInput")
    d_res = nc.dram_tensor("res", (t_steps * P, D), f32, kind="ExternalOutput")

    with tile.TileContext(nc) as tc, ExitStack() as ctx:
        const = ctx.enter_context(tc.tile_pool(name="const", bufs=1))
        act = ctx.enter_context(tc.tile_pool(name="act", bufs=2))
        ps = ctx.enter_context(tc.tile_pool(name="ps", bufs=1, space="PSUM"))

        _gc = [0]

        def gload(shape, dtype, src, name=None):
            _gc[0] += 1
            t = const.tile(shape, dtype, name=name or f"cst{_gc[0]}")
            nc.gpsimd.dma_start(t, src)
            return t

        w8hh0 = gload([P, KP, 2, 3 * H], f8, d_w8hh0.rearrange("k p j n -> p k j n"))
        w8ih1 = gload([P, KP, 2, 3 * H], f8, d_w8ih1.rearrange("k p j n -> p k j n"))
        w8hh1 = gload([P, KP, 2, 3 * H], f8, d_w8hh1.rearrange("k p j n -> p k j n"))
        w8x = gload([P, 1, 2, 3 * H], f8, d_w8x.rearrange("k p j n -> p k j n"))
        w8fc = gload([P, KP, 2, D], f8, d_w8fc.rearrange("k p j n -> p k j n"))
        const0 = gload([P, 4 * H], bf16, d_const0[:])
        const1 = gload([P, 4 * H], bf16, d_const1[:])
        constfc = gload([P, D], bf16, d_constfc[:])
        ident = gload([P, P], bf16, d_ident[:])
        h0 = gload([P, H], f32, d_h0[:])
        h1 = gload([P, H], f32, d_h1[:])
        # fixed-address recurrent fp8 stationary tiles (split lo/hi for
        # fine-grained deps: pairs 0-1 in a, 2-3 in b)
        h0t8a_st = gload([P, 4, P], f8, d_h0t8[:, 0:4, :], name="h0t8a_st")
        h0t8b_st = gload([P, 4, P], f8, d_h0t8[:, 4:8, :], name="h0t8b_st")
        h1t8a_st = gload([P, 4, P], f8, d_h1t8[:, 0:4, :], name="h1t8a_st")
        h1t8b_st = gload([P, 4, P], f8, d_h1t8[:, 4:8, :], name="h1t8b_st")
        xt8_st = gload([P, 2, P], f8, d_xt8[:], name="xt8_st")

        GATES = ("r", "z", "hn", "in")
        GOFF = {"r": 0, "z": 1, "hn": 2, "in": 3}

        state = {"h0t8": (h0t8a_st, h0t8b_st),
                 "h1t8": (h1t8a_st, h1t8b_st),
                 "xt8": xt8_st}

        def pair_ap(t8pair, kt):
            a, b = t8pair
            return (a if kt < 2 else b)[:, 2 * (kt % 2):2 * (kt % 2) + 2, :]

        def gemm_layer(lid, hh_t8, w_hh, ih_t8, n_ih_pairs, w_ih, cst):
            """Emit one GRU layer's gemms.  Returns psum tiles
            {(gate, v): tile}.  Banks: I-const, then hh pairs (r,z,hn),
            then ih pairs (r,z,in)."""
            psg = {}
            for v in (0, 1):
                for g in GATES:
                    pt = ps.tile([P, 512], f32, tag=f"{g}{v}")
                    psg[(g, v)] = pt
                    nc.tensor.matmul(
                        pt, ident,
                        cst[:, GOFF[g] * H + v * 512:GOFF[g] * H + (v + 1) * 512],
                        start=True, stop=False)
                for k in range(KP):
                    lhsT = pair_ap(hh_t8, k)
                    for g, nb in (("r", 0), ("z", H), ("hn", 2 * H)):
                        nc.tensor.matmul(
                            psg[(g, v)], lhsT,
                            w_hh[:, k, :, nb + v * 512:nb + (v + 1) * 512],
                            start=False, stop=(g == "hn" and k == KP - 1),
                            perf_mode=DR)
                for k in range(n_ih_pairs):
                    if n_ih_pairs == 1:
                        lhsT = ih_t8[:, 0:2, :]
                    else:
                        lhsT = pair_ap(ih_t8, k)
                    for g, nb in (("r", 0), ("z", H), ("in", 2 * H)):
                        nc.tensor.matmul(
                            psg[(g, v)], lhsT,
                            w_ih[:, k, :, nb + v * 512:nb + (v + 1) * 512],
                            start=False, stop=k == n_ih_pairs - 1,
                            perf_mode=DR)
            return psg

        def cell(lid, psg, h, t8_out, last_of_body, transpose_engine):
            """GRU cell from psum gates; updates h (f32, in place) and
            writes fp8 transposed pairs into t8_out (a, b)."""
            def ctile(shape, dt, tg, bufs=1):
                return act.tile(shape, dt, tag=tg, bufs=bufs, name=f"cl_{tg}")
            r_sb = [ctile([P, 512], f32, f"r{lid}{v}") for v in (0, 1)]
            z_sb = [ctile([P, 512], f32, f"z{lid}{v}") for v in (0, 1)]
            g1 = [ctile([P, 512], f32, f"a{lid}{v}") for v in (0, 1)]
            npre = [ctile([P, 512], f32, f"b{lid}{v}") for v in (0, 1)]
            n_t = [ctile([P, 512], f32, f"n{lid}{v}") for v in (0, 1)]
            t1 = [ctile([P, 512], f32, f"a{lid}{v}") for v in (0, 1)]
            t2 = [ctile([P, 512], f32, f"b{lid}{v}") for v in (0, 1)]
            hbf = act.tile([P, H], bf16, tag=f"hbf{lid}", bufs=2,
                           name=f"hbf{lid}")
            htbf = [ctile([P, 4, P], bf16, f"htbf{lid}{v}", bufs=2)
                    for v in (0, 1)]
            for v in (0, 1):
                nc.scalar.activation(r_sb[v], psg[("r", v)], AF.Sigmoid,
                                     scale=1.0 / SC)
                nc.scalar.activation(z_sb[v], psg[("z", v)], AF.Sigmoid,
                                     scale=1.0 / SC)
            for v in (0, 1):
                nc.vector.tensor_mul(g1[v], r_sb[v], psg[("hn", v)])
                nc.vector.tensor_add(npre[v], g1[v], psg[("in", v)])
                nc.scalar.activation(n_t[v], npre[v], AF.Tanh, scale=1.0 / SC)
            for v in (0, 1):
                hs = h[:, v * 512:(v + 1) * 512]
                nc.gpsimd.tensor_sub(t1[v], hs, n_t[v])
                nc.gpsimd.tensor_mul(t2[v], z_sb[v], t1[v])
                nc.vector.tensor_add(hs, t2[v], n_t[v])
                nc.vector.tensor_copy(hbf[:, v * 512:(v + 1) * 512], hs)
                transpose_engine.dma_start_transpose(
                    htbf[v], hbf[:, v * 512:(v + 1) * 512])
            if last_of_body:
                out = t8_out
            else:
                out = (act.tile([P, 4, P], f8, tag=f"t8_{lid}a", bufs=2,
                                name=f"t8_{lid}a"),
                       act.tile([P, 4, P], f8, tag=f"t8_{lid}b", bufs=2,
                                name=f"t8_{lid}b"))
            for v in (0, 1):
                nc.scalar.mul(out[v], htbf[v], SH)
            return out

        def emit_step(res_row0, last_of_body):
            cur_h0t8 = state["h0t8"]
            cur_h1t8 = state["h1t8"]
            cur_xt8 = state["xt8"]

            # ---- Layer 0 ----
            psg0 = gemm_layer(0, cur_h0t8, w8hh0, cur_xt8, 1, w8x, const0)
            new_h0t8 = cell(0, psg0, h0, (h0t8a_st, h0t8b_st) if last_of_body
                            else None, last_of_body, nc.sync)

            # ---- Layer 1 ----
            psg1 = gemm_layer(1, cur_h1t8, w8hh1, new_h0t8, KP, w8ih1, const1)
            new_h1t8 = cell(1, psg1, h1, (h1t8a_st, h1t8b_st) if last_of_body
                            else None, last_of_body, nc.sync)

            # ---- FC ----
            ps_fc = ps.tile([P, 512], f32, tag="r0")
            nc.tensor.matmul(ps_fc[:, :D], ident, constfc, start=True,
                             stop=False)
            for k in range(KP):
                nc.tensor.matmul(ps_fc[:, :D], pair_ap(new_h1t8, k),
                                 w8fc[:, k, :, :], start=False,
                                 stop=k == KP - 1, perf_mode=DR)

            # ---- activations / softmax ----
            ALU = mybir.AluOpType
            xf = act.tile([P, D], f32, tag="xf", bufs=2)
            sigp = act.tile([P, 47], f32, tag="sigp", bufs=2)
            sign = act.tile([P, 47], f32, tag="sign", bufs=2)
            s12 = act.tile([P, 2], f32, tag="s12", bufs=2)
            r12 = act.tile([P, 2], f32, tag="r12", bufs=2)
            nc.scalar.activation(xf[:, 47:D], ps_fc[:, 47:D], AF.Sigmoid,
                                 scale=1.0 / SC)
            nc.scalar.activation(sigp, ps_fc[:, 0:47], AF.Sigmoid,
                                 scale=1.0 / SC)
            nc.scalar.activation(sign, ps_fc[:, 0:47], AF.Sigmoid,
                                 scale=-1.0 / SC)
            nc.vector.reciprocal(sign, sign)
            nc.vector.scalar_tensor_tensor(
                xf[:, 0:32], sigp[:, 0:32], 1.0, sign[:, 0:32],
                ALU.mult, ALU.mult, accum_out=s12[:, 0:1])
            nc.vector.scalar_tensor_tensor(
                xf[:, 32:47], sigp[:, 32:47], 1.0, sign[:, 32:47],
                ALU.mult, ALU.mult, accum_out=s12[:, 1:2])
            nc.vector.reciprocal(r12, s12)
            nc.vector.tensor_scalar_mul(xf[:, 0:32], xf[:, 0:32], r12[:, 0:1])
            nc.vector.tensor_scalar_mul(xf[:, 32:47], xf[:, 32:47], r12[:, 1:2])
            nc.gpsimd.dma_start(d_res[bass.ds(res_row0, P), :], xf)

            # ---- x -> fp8 pair tile for next step ----
            xbf = act.tile([P, D], bf16, tag="xbf", bufs=2)
            nc.vector.tensor_copy(xbf, xf)
            xtbf = act.tile([P, 2, P], bf16, tag="xtbf", bufs=2)
            nc.scalar.dma_start_transpose(xtbf, xbf)
            if last_of_body:
                xt8 = xt8_st
            else:
                xt8 = act.tile([P, 2, P], f8, tag="xt8", bufs=2)
            nc.scalar.mul(xt8, xtbf, SH)

            state["h0t8"], state["h1t8"], state["xt8"] = (
                new_h0t8, new_h1t8, xt8)

        if full_unroll:
            for t in range(t_steps):
                emit_step(t * P, last_of_body=False)
        else:
            et = mybir.EngineType
            with tc.For_i(0, n_iter, 1,
                          hint_engines=(et.PE, et.DVE, et.Activation,
                                        et.SP, et.Pool)) as iv:
                row_base = iv * (unroll * P)
                for j in range(unroll):
                    emit_step(row_base + j * P, last_of_body=(j == unroll - 1))

    _dedupe_ldweights(nc, mybir)
    nc.finalize()
    return nc


def _dedupe_ldweights(nc, mybir):
    """Drop redundant back-to-back Ldweights of the same stationary tile."""
    import orjson
    removed = 0
    for func in nc.m.functions:
        for blk in func.blocks:
            last_key = None
            kept = []
            blk_removed = 0
            for inst in blk.instructions:
                if getattr(inst, "engine", None) == mybir.EngineType.PE:
                    d = orjson.loads(mybir.instruction_to_pretty_json_string(inst))
                    op = d.get("opcode")
                    if op == "Ldweights":
                        si = d.get("sync_info") or {}
                        key = orjson.dumps(
                            (d.get("ins"), d.get("tile_position"),
                             d.get("tile_size"), d.get("perf_mode"),
                             d.get("is_transpose")))
                        if (key == last_key and not si.get("on_wait")
                                and not si.get("on_update")):
                            removed += 1
                            blk_removed += 1
                            continue
                        last_key = key
                kept.append(inst)
            if blk_removed:
                blk.instructions[:] = kept
    return removed


def _host_prep(inputs):
    """Build per-core input maps."""
    bf = ml_dtypes.bfloat16
    e4 = ml_dtypes.float8_e4m3
    embed = np.ascontiguousarray(np.asarray(inputs["embed"], dtype=np.float32))
    dynamics = np.asarray(inputs["dynamics"], dtype=np.float32)
    W_ih0 = np.asarray(inputs["W_ih0"], dtype=np.float32)
    W_hh0 = np.asarray(inputs["W_hh0"], dtype=np.float32)
    b_ih0 = np.asarray(inputs["b_ih0"], dtype=np.float32)
    b_hh0 = np.asarray(inputs["b_hh0"], dtype=np.float32)
    W_ih1 = np.asarray(inputs["W_ih1"], dtype=np.float32)
    W_hh1 = np.asarray(inputs["W_hh1"], dtype=np.float32)
    b_ih1 = np.asarray(inputs["b_ih1"], dtype=np.float32)
    b_hh1 = np.asarray(inputs["b_hh1"], dtype=np.float32)
    W_fc = np.asarray(inputs["W_fc"], dtype=np.float32)
    b_fc = np.asarray(inputs["b_fc"], dtype=np.float32)

    glob = embed[:, :H]
    h0i = embed[:, H:2 * H]
    h1i = embed[:, 2 * H:3 * H]
    x0 = dynamics[:, 0, :]

    c0 = (glob.astype(np.float64) @ W_ih0[:, :H].T.astype(np.float64)).astype(np.float32)
    c0 += b_ih0

    def pairize(wT, kp):
        # wT [K, N] -> [kp, P, 2, N] fp8, x SW
        K, N = wT.shape
        assert K == kp * 2 * P
        w8 = np.asarray(wT * SW, dtype=e4)
        return np.ascontiguousarray(w8.reshape(kp, 2, P, N).transpose(0, 2, 1, 3))

    def bcast(row):
        return np.broadcast_to(row, (P, row.shape[0]))

    const1 = np.concatenate([
        bcast((b_ih1 + b_hh1)[:H]),
        bcast((b_ih1 + b_hh1)[H:2 * H]),
        bcast(b_hh1[2 * H:]),
        bcast(b_ih1[2 * H:]),
    ], axis=1) * SC

    shared = {
        "w8hh0": pairize(np.ascontiguousarray(W_hh0.T), KP),
        "w8ih1": pairize(np.ascontiguousarray(W_ih1.T), KP),
        "w8hh1": pairize(np.ascontiguousarray(W_hh1.T), KP),
        "w8x": pairize(np.ascontiguousarray(W_ih0[:, H:].T), 1),
        "w8fc": pairize(np.ascontiguousarray(W_fc.T), KP),
        "const1": np.ascontiguousarray(const1).astype(bf),
        "constfc": np.ascontiguousarray(bcast(b_fc) * SC).astype(bf),
        "ident": np.eye(P, dtype=np.float32).astype(bf),
    }

    def t8(hslice, nslots):
        # [P(batch), nslots*P(features)] -> [P(part=feat within slot), slot, P(batch)]
        hT = np.ascontiguousarray(hslice.T)  # [F, P]
        return np.ascontiguousarray(
            np.asarray(hT * SH, dtype=e4).reshape(nslots, P, P).transpose(1, 0, 2))

    in_maps = []
    for c in range(NCORES):
        s = slice(c * P, (c + 1) * P)
        m = dict(shared)
        m["h0_init"] = np.ascontiguousarray(h0i[s])
        m["h1_init"] = np.ascontiguousarray(h1i[s])
        m["h0t8_init"] = t8(h0i[s], 8)
        m["h1t8_init"] = t8(h1i[s], 8)
        m["xt8_init"] = t8(x0[s], 2)
        const0 = np.concatenate([
            c0[s, :H] + b_hh0[:H],
            c0[s, H:2 * H] + b_hh0[H:2 * H],
            np.broadcast_to(b_hh0[2 * H:], (P, H)),
            c0[s, 2 * H:],
        ], axis=1) * SC
        m["const0"] = np.ascontiguousarray(const0).astype(bf)
        in_maps.append(m)
    return in_maps


def _install_neff_cache():
    """Cache walrus-compiled NEFFs keyed by BIR hash."""
    import hashlib
    import shutil
    import concourse.bass_utils as bu
    import concourse.bass2jax as b2j

    if getattr(bu, "_decoder_neff_cache", False):
        return
    orig = bu.compile_bir_kernel

    def cached(bir_json, tmpdir, neff_name="file.neff"):
        try:
            h = hashlib.sha256(bir_json).hexdigest()[:32]
            cdir = os.path.join(os.path.expanduser("~"), ".cache", "bass_neff")
            os.makedirs(cdir, exist_ok=True)
            cpath = os.path.join(cdir, h + ".neff")
            if os.path.exists(cpath):
                dst = os.path.join(tmpdir, "sg00")
                os.makedirs(dst, exist_ok=True)
                out = os.path.join(dst, neff_name)
                shutil.copy(cpath, out)
                return out
            out = orig(bir_json, tmpdir, neff_name)
            shutil.copy(out, cpath)
            return out
        except Exception:
            return orig(bir_json, tmpdir, neff_name)

    bu.compile_bir_kernel = cached
    b2j.compile_bir_kernel = cached
    bu._decoder_neff_cache = True


def _get_nc():
    key = ("v2", T, 8)
    if key not in _prog_cache:
        _prog_cache[key] = _build_program(T, unroll=8)
    return _prog_cache[key]


def kernel(**inputs):
    from concourse.bass_utils import run_bass_kernel_spmd

    _install_neff_cache()
    nc = _get_nc()
    in_maps = _host_prep(inputs)
    out = run_bass_kernel_spmd(nc, in_maps, core_ids=list(range(NCORES)))
    res = np.concatenate(
        [r["res"].reshape(T, P, D).transpose(1, 0, 2) for r in out.results],
        axis=0)
    return np.ascontiguousarray(res, dtype=np.float32)


# revision 10
# speedup vs baseline: 2.2658x; 1.1767x over previous
"""Trainium2 Bass kernel for nn_Decoder_78305843741218.

2-layer GRU decoder, autoregressive over T=256 steps, batch 1024.
Sharding: data-parallel over batch -> 128 samples/core on 8 cores.

v4 design (per core, per step):
  - All big GEMMs in fp8-e4m3 DoubleRow (2 contraction rows/cycle):
    stationary = transposed activations as fp8 pair-tiles [128,2,128]
    (x16), moving = W^T fp8 pair-chunks [128,2,512] (x64).  PSUM holds
    1024x the true scale; descaled free via activation-scale (1/1024).
  - All weights SBUF-resident in fp8 (~10.5 MB) - no HBM streaming.
  - Per-feature constants (glob @ W_ih0[:,:H].T + biases, x1024, bf16)
    enter PSUM via identity-stationary matmuls.
  - Gates processed in 512-wide halves; 8 PSUM banks = {r,z,hn}x{lo,hi}
    + in-lo + in-hi; the FC time-shares the in-lo bank.
  - Software pipeline: the FC + softmax + x->fp8 chain of step t-1 is
    emitted inside step t's PE stream, covered by L0's I+hh gemms, so
    the PE never waits for the autoregressive x loop.
    Program = prologue (step 0, no FC) + For_i(31) x 8 macro-steps +
    7 tail macro-steps + final FC.
  - Recurrent h0/h1 carried in bf16 (validated: rel err unchanged);
    the cell update is h' = z*h + (1-z)*n with z*h and (1-z) computed
    on GpSimd off the critical path; r*ghn / +gin on DVE from PSUM;
    Tanh/Sigmoid on ScalarE (one ACT table set).
  - h^T via one batched DMA transpose per 512-half (SP hwdge queue;
    x's on the ACT queue), then one ScalarE x16 copy to fp8 pairs.
  - Redundant Ldweights deduped post-Tile.

Output is stored T-major (T*128, D) per core; host reassembles.
"""

import os
import numpy as np
import ml_dtypes

H = 1024
D = 256
T = 256
P = 128
NCORES = 8
KP = 4            # fp8 pair k-tiles for H-dim contraction
SW = 64.0         # weight scale in fp8
SH = 16.0         # activation scale in fp8
SC = SW * SH      # psum scale
UNROLL = 8

_prog_cache = {}


def _build_program(t_steps=T, unroll=UNROLL):
    import concourse.bass as bass
    import concourse.bacc as bacc
    import concourse.mybir as mybir
    import concourse.tile as tile
    from contextlib import ExitStack

    f32 = mybir.dt.float32
    bf16 = mybir.dt.bfloat16
    f8 = mybir.dt.float8e4
    AF = mybir.ActivationFunctionType
    ALU = mybir.AluOpType
    DR = mybir.MatmulPerfMode.DoubleRow

    assert t_steps % unroll == 0
    n_iter = t_steps // unroll - 1   # one iteration's worth is peeled

    nc = bacc.Bacc(None, target_bir_lowering=False)

    # ---- I/O ----
    d_w8hh0 = nc.dram_tensor("w8hh0", (KP, P, 2, 3 * H), f8, kind="ExternalInput")
    d_w8ih1 = nc.dram_tensor("w8ih1", (KP, P, 2, 3 * H), f8, kind="ExternalInput")
    d_w8hh1 = nc.dram_tensor("w8hh1", (KP, P, 2, 3 * H), f8, kind="ExternalInput")
    d_w8x = nc.dram_tensor("w8x", (1, P, 2, 3 * H), f8, kind="ExternalInput")
    d_w8fc = nc.dram_tensor("w8fc", (KP, P, 2, D), f8, kind="ExternalInput")
    d_const0 = nc.dram_tensor("const0", (P, 4 * H), bf16, kind="ExternalInput")
    d_const1 = nc.dram_tensor("const1", (P, 4 * H), bf16, kind="ExternalInput")
    d_constfc = nc.dram_tensor("constfc", (P, D), bf16, kind="ExternalInput")
    d_ident = nc.dram_tensor("ident", (P, P), bf16, kind="ExternalInput")
    d_h0 = nc.dram_tensor("h0_init", (P, H), bf16, kind="ExternalInput")
    d_h1 = nc.dram_tensor("h1_init", (P, H), bf16, kind="ExternalInput")
    d_h0t8 = nc.dram_tensor("h0t8_init", (P, 8, P), f8, kind="ExternalInput")
    d_h1t8 = nc.dram_tensor("h1t8_init", (P, 8, P), f8, kind="ExternalInput")
    d_xt8 = nc.dram_tensor("xt8_init", (P, 2, P), f8, kind="ExternalInput")
    d_res = nc.dram_tensor("res", (t_steps * P, D), f32, kind="ExternalOutput")

    with tile.TileContext(nc) as tc, ExitStack() as ctx:
        const = ctx.enter_context(tc.tile_pool(name="const", bufs=1))
        act = ctx.enter_context(tc.tile_pool(name="act", bufs=2))
        ps = ctx.enter_context(tc.tile_pool(name="ps", bufs=1, space="PSUM"))

        _gc = [0]

        def gload(shape, dtype, src, name=None):
            _gc[0] += 1
            t = const.tile(shape, dtype, name=name or f"cst{_gc[0]}")
            nc.gpsimd.dma_start(t, src)
            return t

        w8hh0 = gload([P, KP, 2, 3 * H], f8, d_w8hh0.rearrange("k p j n -> p k j n"))
        w8ih1 = gload([P, KP, 2, 3 * H], f8, d_w8ih1.rearrange("k p j n -> p k j n"))
        w8hh1 = gload([P, KP, 2, 3 * H], f8, d_w8hh1.rearrange("k p j n -> p k j n"))
        w8x = gload([P, 1, 2, 3 * H], f8, d_w8x.rearrange("k p j n -> p k j n"))
        w8fc = gload([P, KP, 2, D], f8, d_w8fc.rearrange("k p j n -> p k j n"))
        const0 = gload([P, 4 * H], bf16, d_const0[:])
        const1 = gload([P, 4 * H], bf16, d_const1[:])
        constfc = gload([P, D], bf16, d_constfc[:])
        ident = gload([P, P], bf16, d_ident[:])
        h0 = gload([P, H], bf16, d_h0[:])
        h1 = gload([P, H], bf16, d_h1[:])
        h0t8a_st = gload([P, 4, P], f8, d_h0t8[:, 0:4, :], name="h0t8a_st")
        h0t8b_st = gload([P, 4, P], f8, d_h0t8[:, 4:8, :], name="h0t8b_st")
        h1t8a_st = gload([P, 4, P], f8, d_h1t8[:, 0:4, :], name="h1t8a_st")
        h1t8b_st = gload([P, 4, P], f8, d_h1t8[:, 4:8, :], name="h1t8b_st")
        xt8_st = gload([P, 2, P], f8, d_xt8[:], name="xt8_st")

        state = {"h0t8": (h0t8a_st, h0t8b_st),
                 "h1t8": (h1t8a_st, h1t8b_st),
                 "xt8": xt8_st}

        def pair_ap(t8pair, kt):
            a, b = t8pair
            return (a if kt < 2 else b)[:, 2 * (kt % 2):2 * (kt % 2) + 2, :]

        # gate -> (const offset, N base in weight cols)
        GOFFS = {"r": (0, 0), "z": (1, H), "hn": (2, 2 * H), "in": (3, 2 * H)}

        def emit_Ihh(psg, v, hh_t8, w_hh, cst, pfx):
            """I-const + hh-pair matmuls for gates r/z/hn of half v."""
            for g in ("r", "z", "hn"):
                pt = ps.tile([P, 512], f32, tag=f"{g}{v}", name=f"ps_{pfx}{g}{v}")
                psg[(g, v)] = pt
                co, _ = GOFFS[g]
                nc.tensor.matmul(
                    pt, ident,
                    cst[:, co * H + v * 512:co * H + (v + 1) * 512],
                    start=True, stop=False)
            for k in range(KP):
                lhsT = pair_ap(hh_t8, k)
                for g in ("r", "z", "hn"):
                    _, nb = GOFFS[g]
                    nc.tensor.matmul(
                        psg[(g, v)], lhsT,
                        w_hh[:, k, :, nb + v * 512:nb + (v + 1) * 512],
                        start=False, stop=(g == "hn" and k == KP - 1),
                        perf_mode=DR)

        def emit_ih(psg, ih_t8, n_pairs, w_ih, cst, pfx, with_in=True):
            """ih-pair matmuls for gates r/z(/in) of both halves; the
            'in' bank gets its I-const here too (it time-shares with
            the FC bank)."""
            for v in (0, 1):
                if with_in:
                    pt = ps.tile([P, 512], f32, tag=f"in{v}", name=f"ps_{pfx}in{v}")
                    psg[("in", v)] = pt
                    co, _ = GOFFS["in"]
                    nc.tensor.matmul(
                        pt, ident,
                        cst[:, co * H + v * 512:co * H + (v + 1) * 512],
                        start=True, stop=False)
                for k in range(n_pairs):
                    lhsT = (ih_t8[:, 0:2, :] if n_pairs == 1
                            else pair_ap(ih_t8, k))
                    for g in ("r", "z", "in"):
                        _, nb = GOFFS[g]
                        nc.tensor.matmul(
                            psg[(g, v)], lhsT,
                            w_ih[:, k, :, nb + v * 512:nb + (v + 1) * 512],
                            start=False, stop=k == n_pairs - 1,
                            perf_mode=DR)

        def cell(lid, psg, h, t8_state, last_of_body):
            """GRU cell: h (bf16, in place) and fp8 transposed pairs."""
            def ctile(shape, dt, tg, bufs=1):
                return act.tile(shape, dt, tag=tg, bufs=bufs, name=f"cl_{tg}")
            r_sb = [ctile([P, 512], bf16, f"r{lid}{v}") for v in (0, 1)]
            z_sb = [ctile([P, 512], bf16, f"z{lid}{v}") for v in (0, 1)]
            wm = [ctile([P, 512], bf16, f"w{lid}{v}") for v in (0, 1)]
            am = [ctile([P, 512], bf16, f"am{lid}{v}") for v in (0, 1)]
            g1 = [ctile([P, 512], f32, f"a{lid}{v}") for v in (0, 1)]
            npre = [ctile([P, 512], f32, f"b{lid}{v}") for v in (0, 1)]
            n_t = [ctile([P, 512], bf16, f"n{lid}{v}") for v in (0, 1)]
            u = [ctile([P, 512], bf16, f"u{lid}{v}") for v in (0, 1)]
            htbf = [ctile([P, 4, P], bf16, f"htbf{lid}{v}", bufs=2)
                    for v in (0, 1)]
            if last_of_body:
                out = t8_state
            else:
                out = (act.tile([P, 4, P], f8, tag=f"t8_{lid}a", bufs=2,
                                name=f"t8_{lid}a"),
                       act.tile([P, 4, P], f8, tag=f"t8_{lid}b", bufs=2,
                                name=f"t8_{lid}b"))
            for v in (0, 1):
                nc.scalar.activation(r_sb[v], psg[("r", v)], AF.Sigmoid,
                                     scale=1.0 / SC)
                nc.scalar.activation(z_sb[v], psg[("z", v)], AF.Sigmoid,
                                     scale=1.0 / SC)
                nc.gpsimd.tensor_scalar(wm[v], z_sb[v], -1.0, 1.0,
                                        ALU.mult, ALU.add)
                nc.gpsimd.tensor_mul(am[v], z_sb[v], h[:, v * 512:(v + 1) * 512])
            for v in (0, 1):
                nc.vector.tensor_mul(g1[v], r_sb[v], psg[("hn", v)])
                nc.vector.tensor_add(npre[v], g1[v], psg[("in", v)])
            for v in (0, 1):
                nc.scalar.activation(n_t[v], npre[v], AF.Tanh, scale=1.0 / SC)
            for v in (0, 1):
                nc.vector.tensor_mul(u[v], wm[v], n_t[v])
                nc.vector.tensor_add(h[:, v * 512:(v + 1) * 512], am[v], u[v])
                nc.sync.dma_start_transpose(htbf[v], h[:, v * 512:(v + 1) * 512])
            for v in (0, 1):
                nc.scalar.mul(out[v], htbf[v], SH)
            return out

        def emit_fc(h1t8_cur, res_row, make_xt8):
            """FC + softmax/sigmoid + res store (+ x->fp8 pair tile).
            Time-shares the in-lo PSUM bank."""
            ps_fc = ps.tile([P, 512], f32, tag="in0", name="ps_fc")
            nc.tensor.matmul(ps_fc[:, :D], ident, constfc, start=True,
                             stop=False)
            for k in range(KP):
                nc.tensor.matmul(ps_fc[:, :D], pair_ap(h1t8_cur, k),
                                 w8fc[:, k, :, :], start=False,
                                 stop=k == KP - 1, perf_mode=DR)
            xf = act.tile([P, D], f32, tag="xf", bufs=2, name="xf")
            sigp = act.tile([P, 47], f32, tag="sigp", bufs=2, name="sigp")
            sign = act.tile([P, 47], f32, tag="sign", bufs=2, name="sign")
            s12 = act.tile([P, 2], f32, tag="s12", bufs=2, name="s12")
            r12 = act.tile([P, 2], f32, tag="r12", bufs=2, name="r12")
            nc.scalar.activation(xf[:, 47:D], ps_fc[:, 47:D], AF.Sigmoid,
                                 scale=1.0 / SC)
            nc.scalar.activation(sigp, ps_fc[:, 0:47], AF.Sigmoid,
                                 scale=1.0 / SC)
            nc.scalar.activation(sign, ps_fc[:, 0:47], AF.Sigmoid,
                                 scale=-1.0 / SC)
            nc.vector.reciprocal(sign, sign)
            nc.vector.scalar_tensor_tensor(
                xf[:, 0:32], sigp[:, 0:32], 1.0, sign[:, 0:32],
                ALU.mult, ALU.mult, accum_out=s12[:, 0:1])
            nc.vector.scalar_tensor_tensor(
                xf[:, 32:47], sigp[:, 32:47], 1.0, sign[:, 32:47],
                ALU.mult, ALU.mult, accum_out=s12[:, 1:2])
            nc.vector.reciprocal(r12, s12)
            nc.vector.tensor_scalar_mul(xf[:, 0:32], xf[:, 0:32], r12[:, 0:1])
            nc.vector.tensor_scalar_mul(xf[:, 32:47], xf[:, 32:47], r12[:, 1:2])
            nc.gpsimd.dma_start(d_res[bass.ds(res_row, P), :], xf)
            if not make_xt8:
                return None
            xbf = act.tile([P, D], bf16, tag="xbf", bufs=2, name="xbf")
            nc.vector.tensor_copy(xbf, xf)
            xtbf = act.tile([P, 2, P], bf16, tag="xtbf", bufs=2, name="xtbf")
            nc.scalar.dma_start_transpose(xtbf, xbf)
            xt8 = act.tile([P, 2, P], f8, tag="xt8", bufs=2, name="xt8")
            nc.scalar.mul(xt8, xtbf, SH)
            return xt8

        def emit_step(fc_row, last_of_body, do_fc):
            cur_h0t8 = state["h0t8"]
            cur_h1t8 = state["h1t8"]

            psg0 = {}
            # A: L0 I+hh lo
            emit_Ihh(psg0, 0, cur_h0t8, w8hh0, const0, "l0")
            # B: FC of the previous step (+ x -> xt8 chain)
            if do_fc:
                state["xt8"] = emit_fc(cur_h1t8, fc_row, make_xt8=True)
            # C: L0 I+hh hi
            emit_Ihh(psg0, 1, cur_h0t8, w8hh0, const0, "l0")
            # D: L0 x-gemm (r/z/in both halves; allocates in-banks)
            emit_ih(psg0, state["xt8"], 1, w8x, const0, "l0")
            # E: L1 I+hh both halves
            psg1 = {}
            emit_Ihh(psg1, 0, cur_h1t8, w8hh1, const1, "l1")
            emit_Ihh(psg1, 1, cur_h1t8, w8hh1, const1, "l1")
            # F: cell 0 (non-PE)
            new_h0t8 = cell(0, psg0, h0, (h0t8a_st, h0t8b_st), last_of_body)
            # G: L1 ih (r/z/in both halves)
            emit_ih(psg1, new_h0t8, KP, w8ih1, const1, "l1")
            # H: cell 1
            new_h1t8 = cell(1, psg1, h1, (h1t8a_st, h1t8b_st), last_of_body)

            state["h0t8"] = new_h0t8
            state["h1t8"] = new_h1t8

        # ---- prologue: step 0 (x from init, no FC) ----
        emit_step(fc_row=0, last_of_body=True, do_fc=False)

        # ---- main loop: iterations cover steps 1..(n_iter*unroll) ----
        et = mybir.EngineType
        with tc.For_i(0, n_iter, 1,
                      hint_engines=(et.PE, et.DVE, et.Activation,
                                    et.SP, et.Pool)) as iv:
            row_base = iv * (unroll * P)
            for j in range(unroll):
                emit_step(fc_row=row_base + j * P,
                          last_of_body=(j == unroll - 1), do_fc=True)

        # ---- tail: steps (n_iter*unroll + 1) .. (t_steps - 1) ----
        t0 = n_iter * unroll    # first tail fc row
        for m in range(unroll - 1):
            emit_step(fc_row=(t0 + m) * P, last_of_body=False, do_fc=True)
        # final FC for the last step
        emit_fc(state["h1t8"], (t_steps - 1) * P, make_xt8=False)

    _dedupe_ldweights(nc, mybir)
    nc.finalize()
    return nc


def _dedupe_ldweights(nc, mybir):
    """Drop redundant back-to-back Ldweights of the same stationary tile."""
    import orjson
    removed = 0
    for func in nc.m.functions:
        for blk in func.blocks:
            last_key = None
            kept = []
            blk_removed = 0
            for inst in blk.instructions:
                if getattr(inst, "engine", None) == mybir.EngineType.PE:
                    d = orjson.loads(mybir.instruction_to_pretty_json_string(inst))
                    op = d.get("opcode")
                    if op == "Ldweights":
                        si = d.get("sync_info") or {}
                        key = orjson.dumps(
                            (d.get("ins"), d.get("tile_position"),
                             d.get("tile_size"), d.get("perf_mode"),
                             d.get("is_transpose")))
                        if (key == last_key and not si.get("on_wait")
                                and not si.get("on_update")):
                            removed += 1
                            blk_removed += 1
                            continue
                        last_key = key
                kept.append(inst)
            if blk_removed:
                blk.instructions[:] = kept
    return removed


def _host_prep(inputs):
    """Build per-core input maps."""
    bf = ml_dtypes.bfloat16
    e4 = ml_dtypes.float8_e4m3
    embed = np.ascontiguousarray(np.asarray(inputs["embed"], dtype=np.float32))
    dynamics = np.asarray(inputs["dynamics"], dtype=np.float32)
    W_ih0 = np.asarray(inputs["W_ih0"], dtype=np.float32)
    W_hh0 = np.asarray(inputs["W_hh0"], dtype=np.float32)
    b_ih0 = np.asarray(inputs["b_ih0"], dtype=np.float32)
    b_hh0 = np.asarray(inputs["b_hh0"], dtype=np.float32)
    W_ih1 = np.asarray(inputs["W_ih1"], dtype=np.float32)
    W_hh1 = np.asarray(inputs["W_hh1"], dtype=np.float32)
    b_ih1 = np.asarray(inputs["b_ih1"], dtype=np.float32)
    b_hh1 = np.asarray(inputs["b_hh1"], dtype=np.float32)
    W_fc = np.asarray(inputs["W_fc"], dtype=np.float32)
    b_fc = np.asarray(inputs["b_fc"], dtype=np.float32)

    glob = embed[:, :H]
    h0i = embed[:, H:2 * H]
    h1i = embed[:, 2 * H:3 * H]
    x0 = dynamics[:, 0, :]

    c0 = (glob.astype(np.float64) @ W_ih0[:, :H].T.astype(np.float64)).astype(np.float32)
    c0 += b_ih0

    def pairize(wT, kp):
        K, N = wT.shape
        assert K == kp * 2 * P
        w8 = np.asarray(wT * SW, dtype=e4)
        return np.ascontiguousarray(w8.reshape(kp, 2, P, N).transpose(0, 2, 1, 3))

    def bcast(row):
        return np.broadcast_to(row, (P, row.shape[0]))

    const1 = np.concatenate([
        bcast((b_ih1 + b_hh1)[:H]),
        bcast((b_ih1 + b_hh1)[H:2 * H]),
        bcast(b_hh1[2 * H:]),
        bcast(b_ih1[2 * H:]),
    ], axis=1) * SC

    shared = {
        "w8hh0": pairize(np.ascontiguousarray(W_hh0.T), KP),
        "w8ih1": pairize(np.ascontiguousarray(W_ih1.T), KP),
        "w8hh1": pairize(np.ascontiguousarray(W_hh1.T), KP),
        "w8x": pairize(np.ascontiguousarray(W_ih0[:, H:].T), 1),
        "w8fc": pairize(np.ascontiguousarray(W_fc.T), KP),
        "const1": np.ascontiguousarray(const1).astype(bf),
        "constfc": np.ascontiguousarray(bcast(b_fc) * SC).astype(bf),
        "ident": np.eye(P, dtype=np.float32).astype(bf),
    }

    def t8(hslice, nslots):
        hT = np.ascontiguousarray(np.asarray(hslice, np.float32).T)
        return np.ascontiguousarray(
            np.asarray(hT * SH, dtype=e4).reshape(nslots, P, P).transpose(1, 0, 2))

    in_maps = []
    for c in range(NCORES):
        s = slice(c * P, (c + 1) * P)
        m = dict(shared)
        h0bf = h0i[s].astype(bf)
        h1bf = h1i[s].astype(bf)
        m["h0_init"] = np.ascontiguousarray(h0bf)
        m["h1_init"] = np.ascontiguousarray(h1bf)
        m["h0t8_init"] = t8(h0bf.astype(np.float32), 8)
        m["h1t8_init"] = t8(h1bf.astype(np.float32), 8)
        m["xt8_init"] = t8(x0[s], 2)
        const0 = np.concatenate([
            c0[s, :H] + b_hh0[:H],
            c0[s, H:2 * H] + b_hh0[H:2 * H],
            np.broadcast_to(b_hh0[2 * H:], (P, H)),
            c0[s, 2 * H:],
        ], axis=1) * SC
        m["const0"] = np.ascontiguousarray(const0).astype(bf)
        in_maps.append(m)
    return in_maps


def _install_neff_cache():
    """Cache walrus-compiled NEFFs keyed by BIR hash."""
    import hashlib
    import shutil
    import concourse.bass_utils as bu
    import concourse.bass2jax as b2j

    if getattr(bu, "_decoder_neff_cache", False):
        return
    orig = bu.compile_bir_kernel

    def cached(bir_json, tmpdir, neff_name="file.neff"):
        try:
            h = hashlib.sha256(bir_json).hexdigest()[:32]
            cdir = os.path.join(os.path.expanduser("~"), ".cache", "bass_neff")
            os.makedirs(cdir, exist_ok=True)
            cpath = os.path.join(cdir, h + ".neff")
            if os.path.exists(cpath):
                dst = os.path.join(tmpdir, "sg00")
                os.makedirs(dst, exist_ok=True)
                out = os.path.join(dst, neff_name)
                shutil.copy(cpath, out)
                return out
            out = orig(bir_json, tmpdir, neff_name)
            shutil.copy(out, cpath)
            return out
        except Exception:
            return orig(bir_json, tmpdir, neff_name)

    bu.compile_bir_kernel = cached
    b2j.compile_bir_kernel = cached
    bu._decoder_neff_cache = True


def _get_nc():
    key = ("v4", T, UNROLL)
    if key not in _prog_cache:
        _prog_cache[key] = _build_program(T, unroll=UNROLL)
    return _prog_cache[key]


def kernel(**inputs):
    from concourse.bass_utils import run_bass_kernel_spmd

    _install_neff_cache()
    nc = _get_nc()
    in_maps = _host_prep(inputs)
    out = run_bass_kernel_spmd(nc, in_maps, core_ids=list(range(NCORES)))
    res = np.concatenate(
        [r["res"].reshape(T, P, D).transpose(1, 0, 2) for r in out.results],
        axis=0)
    return np.ascontiguousarray(res, dtype=np.float32)


# revision 11
# speedup vs baseline: 2.4252x; 1.0703x over previous
"""Trainium2 Bass kernel for nn_Decoder_78305843741218.

2-layer GRU decoder, autoregressive over T=256 steps, batch 1024.
Sharding: data-parallel over batch -> 128 samples/core on 8 cores.

v4 design (per core, per step):
  - All big GEMMs in fp8-e4m3 DoubleRow (2 contraction rows/cycle):
    stationary = transposed activations as fp8 pair-tiles [128,2,128]
    (x16), moving = W^T fp8 pair-chunks [128,2,512] (x64).  PSUM holds
    1024x the true scale; descaled free via activation-scale (1/1024).
  - All weights SBUF-resident in fp8 (~10.5 MB) - no HBM streaming.
  - Per-feature constants (glob @ W_ih0[:,:H].T + biases, x1024, bf16)
    enter PSUM via identity-stationary matmuls.
  - Gates processed in 512-wide halves; 8 PSUM banks = {r,z,hn}x{lo,hi}
    + in-lo + in-hi; the FC time-shares the in-lo bank.
  - Software pipeline: the FC + softmax + x->fp8 chain of step t-1 is
    emitted inside step t's PE stream, covered by L0's I+hh gemms, so
    the PE never waits for the autoregressive x loop.
    Program = prologue (step 0, no FC) + For_i(31) x 8 macro-steps +
    7 tail macro-steps + final FC.
  - Recurrent h0/h1 carried in bf16 (validated: rel err unchanged);
    the cell update is h' = z*h + (1-z)*n with z*h and (1-z) computed
    on GpSimd off the critical path; r*ghn / +gin on DVE from PSUM;
    Tanh/Sigmoid on ScalarE (one ACT table set).
  - h^T via one batched DMA transpose per 512-half (SP hwdge queue;
    x's on the ACT queue), then one ScalarE x16 copy to fp8 pairs.
  - Redundant Ldweights deduped post-Tile.

Output is stored T-major (T*128, D) per core; host reassembles.
"""

import os
import numpy as np
import ml_dtypes

H = 1024
D = 256
T = 256
P = 128
NCORES = 8
KP = 4            # fp8 pair k-tiles for H-dim contraction
SW = 64.0         # weight scale in fp8
SH = 16.0         # activation scale in fp8
SC = SW * SH      # psum scale
UNROLL = 16

_prog_cache = {}


def _build_program(t_steps=T, unroll=UNROLL):
    import concourse.bass as bass
    import concourse.bacc as bacc
    import concourse.mybir as mybir
    import concourse.tile as tile
    from contextlib import ExitStack

    f32 = mybir.dt.float32
    bf16 = mybir.dt.bfloat16
    f8 = mybir.dt.float8e4
    AF = mybir.ActivationFunctionType
    ALU = mybir.AluOpType
    DR = mybir.MatmulPerfMode.DoubleRow

    assert t_steps % unroll == 0
    n_iter = t_steps // unroll - 1   # one iteration's worth is peeled

    nc = bacc.Bacc(None, target_bir_lowering=False)

    # ---- I/O ----
    d_w8hh0 = nc.dram_tensor("w8hh0", (KP, P, 2, 3 * H), f8, kind="ExternalInput")
    d_w8ih1 = nc.dram_tensor("w8ih1", (KP, P, 2, 3 * H), f8, kind="ExternalInput")
    d_w8hh1 = nc.dram_tensor("w8hh1", (KP, P, 2, 3 * H), f8, kind="ExternalInput")
    d_w8x = nc.dram_tensor("w8x", (1, P, 2, 3 * H), f8, kind="ExternalInput")
    d_w8fc = nc.dram_tensor("w8fc", (KP, P, 2, D), f8, kind="ExternalInput")
    d_const0 = nc.dram_tensor("const0", (P, 4 * H), bf16, kind="ExternalInput")
    d_const1 = nc.dram_tensor("const1", (P, 4 * H), bf16, kind="ExternalInput")
    d_constfc = nc.dram_tensor("constfc", (P, D), bf16, kind="ExternalInput")
    d_ident = nc.dram_tensor("ident", (P, P), bf16, kind="ExternalInput")
    d_h0 = nc.dram_tensor("h0_init", (P, H), bf16, kind="ExternalInput")
    d_h1 = nc.dram_tensor("h1_init", (P, H), bf16, kind="ExternalInput")
    d_h0t8 = nc.dram_tensor("h0t8_init", (P, 8, P), f8, kind="ExternalInput")
    d_h1t8 = nc.dram_tensor("h1t8_init", (P, 8, P), f8, kind="ExternalInput")
    d_xt8 = nc.dram_tensor("xt8_init", (P, 2, P), f8, kind="ExternalInput")
    d_res = nc.dram_tensor("res", (t_steps * P, D), f32, kind="ExternalOutput")

    with tile.TileContext(nc) as tc, ExitStack() as ctx:
        const = ctx.enter_context(tc.tile_pool(name="const", bufs=1))
        act = ctx.enter_context(tc.tile_pool(name="act", bufs=2))
        ps = ctx.enter_context(tc.tile_pool(name="ps", bufs=1, space="PSUM"))

        _gc = [0]

        def gload(shape, dtype, src, name=None):
            _gc[0] += 1
            t = const.tile(shape, dtype, name=name or f"cst{_gc[0]}")
            nc.gpsimd.dma_start(t, src)
            return t

        w8hh0 = gload([P, KP, 2, 3 * H], f8, d_w8hh0.rearrange("k p j n -> p k j n"))
        w8ih1 = gload([P, KP, 2, 3 * H], f8, d_w8ih1.rearrange("k p j n -> p k j n"))
        w8hh1 = gload([P, KP, 2, 3 * H], f8, d_w8hh1.rearrange("k p j n -> p k j n"))
        w8x = gload([P, 1, 2, 3 * H], f8, d_w8x.rearrange("k p j n -> p k j n"))
        w8fc = gload([P, KP, 2, D], f8, d_w8fc.rearrange("k p j n -> p k j n"))
        const0 = gload([P, 4 * H], bf16, d_const0[:])
        const1 = gload([P, 4 * H], bf16, d_const1[:])
        constfc = gload([P, D], bf16, d_constfc[:])
        ident = gload([P, P], bf16, d_ident[:])
        h0 = gload([P, H], bf16, d_h0[:])
        h1 = gload([P, H], bf16, d_h1[:])
        h0t8a_st = gload([P, 4, P], f8, d_h0t8[:, 0:4, :], name="h0t8a_st")
        h0t8b_st = gload([P, 4, P], f8, d_h0t8[:, 4:8, :], name="h0t8b_st")
        h1t8a_st = gload([P, 4, P], f8, d_h1t8[:, 0:4, :], name="h1t8a_st")
        h1t8b_st = gload([P, 4, P], f8, d_h1t8[:, 4:8, :], name="h1t8b_st")
        xt8_st = gload([P, 2, P], f8, d_xt8[:], name="xt8_st")

        state = {"h0t8": (h0t8a_st, h0t8b_st),
                 "h1t8": (h1t8a_st, h1t8b_st),
                 "xt8": xt8_st}

        def pair_ap(t8pair, kt):
            a, b = t8pair
            return (a if kt < 2 else b)[:, 2 * (kt % 2):2 * (kt % 2) + 2, :]

        # gate -> (const offset, N base in weight cols)
        GOFFS = {"r": (0, 0), "z": (1, H), "hn": (2, 2 * H), "in": (3, 2 * H)}

        def emit_Ihh(psg, v, hh_t8, w_hh, cst, pfx):
            """I-const + hh-pair matmuls for gates r/z/hn of half v."""
            for g in ("r", "z", "hn"):
                pt = ps.tile([P, 512], f32, tag=f"{g}{v}", name=f"ps_{pfx}{g}{v}")
                psg[(g, v)] = pt
                co, _ = GOFFS[g]
                nc.tensor.matmul(
                    pt, ident,
                    cst[:, co * H + v * 512:co * H + (v + 1) * 512],
                    start=True, stop=False)
            for k in range(KP):
                lhsT = pair_ap(hh_t8, k)
                for g in ("r", "z", "hn"):
                    _, nb = GOFFS[g]
                    nc.tensor.matmul(
                        psg[(g, v)], lhsT,
                        w_hh[:, k, :, nb + v * 512:nb + (v + 1) * 512],
                        start=False, stop=(g == "hn" and k == KP - 1),
                        perf_mode=DR)

        def emit_ih(psg, ih_t8, n_pairs, w_ih, cst, pfx, with_in=True):
            """ih-pair matmuls for gates r/z(/in) of both halves; the
            'in' bank gets its I-const here too (it time-shares with
            the FC bank)."""
            for v in (0, 1):
                if with_in:
                    pt = ps.tile([P, 512], f32, tag=f"in{v}", name=f"ps_{pfx}in{v}")
                    psg[("in", v)] = pt
                    co, _ = GOFFS["in"]
                    nc.tensor.matmul(
                        pt, ident,
                        cst[:, co * H + v * 512:co * H + (v + 1) * 512],
                        start=True, stop=False)
                for k in range(n_pairs):
                    lhsT = (ih_t8[:, 0:2, :] if n_pairs == 1
                            else pair_ap(ih_t8, k))
                    for g in ("r", "z", "in"):
                        _, nb = GOFFS[g]
                        nc.tensor.matmul(
                            psg[(g, v)], lhsT,
                            w_ih[:, k, :, nb + v * 512:nb + (v + 1) * 512],
                            start=False, stop=k == n_pairs - 1,
                            perf_mode=DR)

        def cell(lid, psg, h, t8_state, last_of_body):
            """GRU cell: h (bf16, in place) and fp8 transposed pairs."""
            def ctile(shape, dt, tg, bufs=1):
                return act.tile(shape, dt, tag=tg, bufs=bufs, name=f"cl_{tg}")
            r_sb = [ctile([P, 512], bf16, f"r{lid}{v}") for v in (0, 1)]
            z_sb = [ctile([P, 512], bf16, f"z{lid}{v}") for v in (0, 1)]
            wm = [ctile([P, 512], bf16, f"w{lid}{v}") for v in (0, 1)]
            am = [ctile([P, 512], bf16, f"am{lid}{v}") for v in (0, 1)]
            g1 = [ctile([P, 512], f32, f"a{lid}{v}") for v in (0, 1)]
            npre = [ctile([P, 512], f32, f"b{lid}{v}") for v in (0, 1)]
            n_t = [ctile([P, 512], bf16, f"n{lid}{v}") for v in (0, 1)]
            u = [ctile([P, 512], bf16, f"u{lid}{v}") for v in (0, 1)]
            htbf = [ctile([P, 4, P], bf16, f"htbf{lid}{v}", bufs=2)
                    for v in (0, 1)]
            if last_of_body:
                out = t8_state
            else:
                out = (act.tile([P, 4, P], f8, tag=f"t8_{lid}a", bufs=2,
                                name=f"t8_{lid}a"),
                       act.tile([P, 4, P], f8, tag=f"t8_{lid}b", bufs=2,
                                name=f"t8_{lid}b"))
            for v in (0, 1):
                nc.scalar.activation(r_sb[v], psg[("r", v)], AF.Sigmoid,
                                     scale=1.0 / SC)
                nc.scalar.activation(z_sb[v], psg[("z", v)], AF.Sigmoid,
                                     scale=1.0 / SC)
                nc.gpsimd.tensor_scalar(wm[v], z_sb[v], -1.0, 1.0,
                                        ALU.mult, ALU.add)
                nc.gpsimd.tensor_mul(am[v], z_sb[v], h[:, v * 512:(v + 1) * 512])
            for v in (0, 1):
                nc.vector.tensor_mul(g1[v], r_sb[v], psg[("hn", v)])
                nc.vector.tensor_add(npre[v], g1[v], psg[("in", v)])
            for v in (0, 1):
                nc.scalar.activation(n_t[v], npre[v], AF.Tanh, scale=1.0 / SC)
            for v in (0, 1):
                nc.vector.tensor_mul(u[v], wm[v], n_t[v])
                nc.vector.tensor_add(h[:, v * 512:(v + 1) * 512], am[v], u[v])
                nc.sync.dma_start_transpose(htbf[v], h[:, v * 512:(v + 1) * 512])
            for v in (0, 1):
                nc.scalar.mul(out[v], htbf[v], SH)
            return out

        def emit_fc(h1t8_cur, res_row, make_xt8):
            """FC + softmax/sigmoid + res store (+ x->fp8 pair tile).
            Time-shares the in-lo PSUM bank."""
            ps_fc = ps.tile([P, 512], f32, tag="in0", name="ps_fc")
            nc.tensor.matmul(ps_fc[:, :D], ident, constfc, start=True,
                             stop=False)
            for k in range(KP):
                nc.tensor.matmul(ps_fc[:, :D], pair_ap(h1t8_cur, k),
                                 w8fc[:, k, :, :], start=False,
                                 stop=k == KP - 1, perf_mode=DR)
            xf = act.tile([P, D], f32, tag="xf", bufs=2, name="xf")
            sigp = act.tile([P, 47], f32, tag="sigp", bufs=2, name="sigp")
            sign = act.tile([P, 47], f32, tag="sign", bufs=2, name="sign")
            s12 = act.tile([P, 2], f32, tag="s12", bufs=2, name="s12")
            r12 = act.tile([P, 2], f32, tag="r12", bufs=2, name="r12")
            nc.scalar.activation(xf[:, 47:D], ps_fc[:, 47:D], AF.Sigmoid,
                                 scale=1.0 / SC)
            nc.scalar.activation(sigp, ps_fc[:, 0:47], AF.Sigmoid,
                                 scale=1.0 / SC)
            nc.scalar.activation(sign, ps_fc[:, 0:47], AF.Sigmoid,
                                 scale=-1.0 / SC)
            nc.vector.reciprocal(sign, sign)
            nc.vector.scalar_tensor_tensor(
                xf[:, 0:32], sigp[:, 0:32], 1.0, sign[:, 0:32],
                ALU.mult, ALU.mult, accum_out=s12[:, 0:1])
            nc.vector.scalar_tensor_tensor(
                xf[:, 32:47], sigp[:, 32:47], 1.0, sign[:, 32:47],
                ALU.mult, ALU.mult, accum_out=s12[:, 1:2])
            nc.vector.reciprocal(r12, s12)
            nc.vector.tensor_scalar_mul(xf[:, 0:32], xf[:, 0:32], r12[:, 0:1])
            nc.vector.tensor_scalar_mul(xf[:, 32:47], xf[:, 32:47], r12[:, 1:2])
            nc.gpsimd.dma_start(d_res[bass.ds(res_row, P), :], xf)
            if not make_xt8:
                return None
            xbf = act.tile([P, D], bf16, tag="xbf", bufs=2, name="xbf")
            nc.vector.tensor_copy(xbf, xf)
            xtbf = act.tile([P, 2, P], bf16, tag="xtbf", bufs=2, name="xtbf")
            nc.scalar.dma_start_transpose(xtbf, xbf)
            xt8 = act.tile([P, 2, P], f8, tag="xt8", bufs=2, name="xt8")
            nc.scalar.mul(xt8, xtbf, SH)
            return xt8

        def emit_step(fc_row, last_of_body, do_fc):
            cur_h0t8 = state["h0t8"]
            cur_h1t8 = state["h1t8"]

            psg0 = {}
            # A: L0 I+hh lo
            emit_Ihh(psg0, 0, cur_h0t8, w8hh0, const0, "l0")
            # B: FC of the previous step (+ x -> xt8 chain)
            if do_fc:
                state["xt8"] = emit_fc(cur_h1t8, fc_row, make_xt8=True)
            # C: L0 I+hh hi
            emit_Ihh(psg0, 1, cur_h0t8, w8hh0, const0, "l0")
            # D: L0 x-gemm (r/z/in both halves; allocates in-banks)
            emit_ih(psg0, state["xt8"], 1, w8x, const0, "l0")
            # E: L1 I+hh both halves
            psg1 = {}
            emit_Ihh(psg1, 0, cur_h1t8, w8hh1, const1, "l1")
            emit_Ihh(psg1, 1, cur_h1t8, w8hh1, const1, "l1")
            # F: cell 0 (non-PE)
            new_h0t8 = cell(0, psg0, h0, (h0t8a_st, h0t8b_st), last_of_body)
            # G: L1 ih (r/z/in both halves)
            emit_ih(psg1, new_h0t8, KP, w8ih1, const1, "l1")
            # H: cell 1
            new_h1t8 = cell(1, psg1, h1, (h1t8a_st, h1t8b_st), last_of_body)

            state["h0t8"] = new_h0t8
            state["h1t8"] = new_h1t8

        # ---- prologue: step 0 (x from init, no FC) ----
        emit_step(fc_row=0, last_of_body=True, do_fc=False)

        # ---- main loop: iterations cover steps 1..(n_iter*unroll) ----
        et = mybir.EngineType
        with tc.For_i(0, n_iter, 1,
                      hint_engines=(et.PE, et.DVE, et.Activation,
                                    et.SP, et.Pool)) as iv:
            row_base = iv * (unroll * P)
            for j in range(unroll):
                emit_step(fc_row=row_base + j * P,
                          last_of_body=(j == unroll - 1), do_fc=True)

        # ---- tail: steps (n_iter*unroll + 1) .. (t_steps - 1) ----
        t0 = n_iter * unroll    # first tail fc row
        for m in range(unroll - 1):
            emit_step(fc_row=(t0 + m) * P, last_of_body=False, do_fc=True)
        # final FC for the last step
        emit_fc(state["h1t8"], (t_steps - 1) * P, make_xt8=False)

    _dedupe_ldweights(nc, mybir)
    nc.finalize()
    return nc


def _dedupe_ldweights(nc, mybir):
    """Drop redundant back-to-back Ldweights of the same stationary tile."""
    import orjson
    removed = 0
    for func in nc.m.functions:
        for blk in func.blocks:
            last_key = None
            kept = []
            blk_removed = 0
            for inst in blk.instructions:
                if getattr(inst, "engine", None) == mybir.EngineType.PE:
                    d = orjson.loads(mybir.instruction_to_pretty_json_string(inst))
                    op = d.get("opcode")
                    if op == "Ldweights":
                        si = d.get("sync_info") or {}
                        key = orjson.dumps(
                            (d.get("ins"), d.get("tile_position"),
                             d.get("tile_size"), d.get("perf_mode"),
                             d.get("is_transpose")))
                        if (key == last_key and not si.get("on_wait")
                                and not si.get("on_update")):
                            removed += 1
                            blk_removed += 1
                            continue
                        last_key = key
                kept.append(inst)
            if blk_removed:
                blk.instructions[:] = kept
    return removed


def _host_prep(inputs):
    """Build per-core input maps."""
    bf = ml_dtypes.bfloat16
    e4 = ml_dtypes.float8_e4m3
    embed = np.ascontiguousarray(np.asarray(inputs["embed"], dtype=np.float32))
    dynamics = np.asarray(inputs["dynamics"], dtype=np.float32)
    W_ih0 = np.asarray(inputs["W_ih0"], dtype=np.float32)
    W_hh0 = np.asarray(inputs["W_hh0"], dtype=np.float32)
    b_ih0 = np.asarray(inputs["b_ih0"], dtype=np.float32)
    b_hh0 = np.asarray(inputs["b_hh0"], dtype=np.float32)
    W_ih1 = np.asarray(inputs["W_ih1"], dtype=np.float32)
    W_hh1 = np.asarray(inputs["W_hh1"], dtype=np.float32)
    b_ih1 = np.asarray(inputs["b_ih1"], dtype=np.float32)
    b_hh1 = np.asarray(inputs["b_hh1"], dtype=np.float32)
    W_fc = np.asarray(inputs["W_fc"], dtype=np.float32)
    b_fc = np.asarray(inputs["b_fc"], dtype=np.float32)

    glob = embed[:, :H]
    h0i = embed[:, H:2 * H]
    h1i = embed[:, 2 * H:3 * H]
    x0 = dynamics[:, 0, :]

    c0 = (glob.astype(np.float64) @ W_ih0[:, :H].T.astype(np.float64)).astype(np.float32)
    c0 += b_ih0

    def pairize(wT, kp):
        K, N = wT.shape
        assert K == kp * 2 * P
        w8 = np.asarray(wT * SW, dtype=e4)
        return np.ascontiguousarray(w8.reshape(kp, 2, P, N).transpose(0, 2, 1, 3))

    def bcast(row):
        return np.broadcast_to(row, (P, row.shape[0]))

    const1 = np.concatenate([
        bcast((b_ih1 + b_hh1)[:H]),
        bcast((b_ih1 + b_hh1)[H:2 * H]),
        bcast(b_hh1[2 * H:]),
        bcast(b_ih1[2 * H:]),
    ], axis=1) * SC

    shared = {
        "w8hh0": pairize(np.ascontiguousarray(W_hh0.T), KP),
        "w8ih1": pairize(np.ascontiguousarray(W_ih1.T), KP),
        "w8hh1": pairize(np.ascontiguousarray(W_hh1.T), KP),
        "w8x": pairize(np.ascontiguousarray(W_ih0[:, H:].T), 1),
        "w8fc": pairize(np.ascontiguousarray(W_fc.T), KP),
        "const1": np.ascontiguousarray(const1).astype(bf),
        "constfc": np.ascontiguousarray(bcast(b_fc) * SC).astype(bf),
        "ident": np.eye(P, dtype=np.float32).astype(bf),
    }

    def t8(hslice, nslots):
        hT = np.ascontiguousarray(np.asarray(hslice, np.float32).T)
        return np.ascontiguousarray(
            np.asarray(hT * SH, dtype=e4).reshape(nslots, P, P).transpose(1, 0, 2))

    in_maps = []
    for c in range(NCORES):
        s = slice(c * P, (c + 1) * P)
        m = dict(shared)
        h0bf = h0i[s].astype(bf)
        h1bf = h1i[s].astype(bf)
        m["h0_init"] = np.ascontiguousarray(h0bf)
        m["h1_init"] = np.ascontiguousarray(h1bf)
        m["h0t8_init"] = t8(h0bf.astype(np.float32), 8)
        m["h1t8_init"] = t8(h1bf.astype(np.float32), 8)
        m["xt8_init"] = t8(x0[s], 2)
        const0 = np.concatenate([
            c0[s, :H] + b_hh0[:H],
            c0[s, H:2 * H] + b_hh0[H:2 * H],
            np.broadcast_to(b_hh0[2 * H:], (P, H)),
            c0[s, 2 * H:],
        ], axis=1) * SC
        m["const0"] = np.ascontiguousarray(const0).astype(bf)
        in_maps.append(m)
    return in_maps


def _install_neff_cache():
    """Cache walrus-compiled NEFFs keyed by BIR hash."""
    import hashlib
    import shutil
    import concourse.bass_utils as bu
    import concourse.bass2jax as b2j

    if getattr(bu, "_decoder_neff_cache", False):
        return
    orig = bu.compile_bir_kernel

    def cached(bir_json, tmpdir, neff_name="file.neff"):
        try:
            h = hashlib.sha256(bir_json).hexdigest()[:32]
            cdir = os.path.join(os.path.expanduser("~"), ".cache", "bass_neff")
            os.makedirs(cdir, exist_ok=True)
            cpath = os.path.join(cdir, h + ".neff")
            if os.path.exists(cpath):
                dst = os.path.join(tmpdir, "sg00")
                os.makedirs(dst, exist_ok=True)
                out = os.path.join(dst, neff_name)
                shutil.copy(cpath, out)
                return out
            out = orig(bir_json, tmpdir, neff_name)
            shutil.copy(out, cpath)
            return out
        except Exception:
            return orig(bir_json, tmpdir, neff_name)

    bu.compile_bir_kernel = cached
    b2j.compile_bir_kernel = cached
    bu._decoder_neff_cache = True


def _get_nc():
    key = ("v4", T, UNROLL)
    if key not in _prog_cache:
        _prog_cache[key] = _build_program(T, unroll=UNROLL)
    return _prog_cache[key]


def kernel(**inputs):
    from concourse.bass_utils import run_bass_kernel_spmd

    _install_neff_cache()
    nc = _get_nc()
    in_maps = _host_prep(inputs)
    out = run_bass_kernel_spmd(nc, in_maps, core_ids=list(range(NCORES)))
    res = np.concatenate(
        [r["res"].reshape(T, P, D).transpose(1, 0, 2) for r in out.results],
        axis=0)
    return np.ascontiguousarray(res, dtype=np.float32)


# revision 12
# speedup vs baseline: 2.4515x; 1.0109x over previous
"""Trainium2 Bass kernel for nn_Decoder_78305843741218.

2-layer GRU decoder, autoregressive over T=256 steps, batch 1024.
Sharding: data-parallel over batch -> 128 samples/core on 8 cores.

v4 design (per core, per step):
  - All big GEMMs in fp8-e4m3 DoubleRow (2 contraction rows/cycle):
    stationary = transposed activations as fp8 pair-tiles [128,2,128]
    (x16), moving = W^T fp8 pair-chunks [128,2,512] (x64).  PSUM holds
    1024x the true scale; descaled free via activation-scale (1/1024).
  - All weights SBUF-resident in fp8 (~10.5 MB) - no HBM streaming.
  - Per-feature constants (glob @ W_ih0[:,:H].T + biases, x1024, bf16)
    enter PSUM via identity-stationary matmuls.
  - Gates processed in 512-wide halves; 8 PSUM banks = {r,z,hn}x{lo,hi}
    + in-lo + in-hi; the FC time-shares the in-lo bank.
  - Software pipeline: the FC + softmax + x->fp8 chain of step t-1 is
    emitted inside step t's PE stream, covered by L0's I+hh gemms, so
    the PE never waits for the autoregressive x loop.
    Program = prologue (step 0, no FC) + For_i(31) x 8 macro-steps +
    7 tail macro-steps + final FC.
  - Recurrent h0/h1 carried in bf16 (validated: rel err unchanged);
    the cell update is h' = z*h + (1-z)*n with z*h and (1-z) computed
    on GpSimd off the critical path; r*ghn / +gin on DVE from PSUM;
    Tanh/Sigmoid on ScalarE (one ACT table set).
  - h^T via one batched DMA transpose per 512-half (SP hwdge queue;
    x's on the ACT queue), then one ScalarE x16 copy to fp8 pairs.
  - Redundant Ldweights deduped post-Tile.

Output is stored T-major (T*128, D) per core; host reassembles.
"""

import os
import numpy as np
import ml_dtypes

H = 1024
D = 256
T = 256
P = 128
NCORES = 8
KP = 4            # fp8 pair k-tiles for H-dim contraction
SW = 64.0         # weight scale in fp8
SH = 16.0         # activation scale in fp8
SC = SW * SH      # psum scale
UNROLL = 32

_prog_cache = {}


def _build_program(t_steps=T, unroll=UNROLL):
    import concourse.bass as bass
    import concourse.bacc as bacc
    import concourse.mybir as mybir
    import concourse.tile as tile
    from contextlib import ExitStack

    f32 = mybir.dt.float32
    bf16 = mybir.dt.bfloat16
    f8 = mybir.dt.float8e4
    AF = mybir.ActivationFunctionType
    ALU = mybir.AluOpType
    DR = mybir.MatmulPerfMode.DoubleRow

    assert t_steps % unroll == 0
    n_iter = t_steps // unroll - 1   # one iteration's worth is peeled

    nc = bacc.Bacc(None, target_bir_lowering=False)

    # ---- I/O ----
    d_w8hh0 = nc.dram_tensor("w8hh0", (KP, P, 2, 3 * H), f8, kind="ExternalInput")
    d_w8ih1 = nc.dram_tensor("w8ih1", (KP, P, 2, 3 * H), f8, kind="ExternalInput")
    d_w8hh1 = nc.dram_tensor("w8hh1", (KP, P, 2, 3 * H), f8, kind="ExternalInput")
    d_w8x = nc.dram_tensor("w8x", (1, P, 2, 3 * H), f8, kind="ExternalInput")
    d_w8fc = nc.dram_tensor("w8fc", (KP, P, 2, D), f8, kind="ExternalInput")
    d_const0 = nc.dram_tensor("const0", (P, 4 * H), bf16, kind="ExternalInput")
    d_const1 = nc.dram_tensor("const1", (P, 4 * H), bf16, kind="ExternalInput")
    d_constfc = nc.dram_tensor("constfc", (P, D), bf16, kind="ExternalInput")
    d_ident = nc.dram_tensor("ident", (P, P), bf16, kind="ExternalInput")
    d_h0 = nc.dram_tensor("h0_init", (P, H), bf16, kind="ExternalInput")
    d_h1 = nc.dram_tensor("h1_init", (P, H), bf16, kind="ExternalInput")
    d_h0t8 = nc.dram_tensor("h0t8_init", (P, 8, P), f8, kind="ExternalInput")
    d_h1t8 = nc.dram_tensor("h1t8_init", (P, 8, P), f8, kind="ExternalInput")
    d_xt8 = nc.dram_tensor("xt8_init", (P, 2, P), f8, kind="ExternalInput")
    d_res = nc.dram_tensor("res", (t_steps * P, D), f32, kind="ExternalOutput")

    with tile.TileContext(nc) as tc, ExitStack() as ctx:
        const = ctx.enter_context(tc.tile_pool(name="const", bufs=1))
        act = ctx.enter_context(tc.tile_pool(name="act", bufs=2))
        ps = ctx.enter_context(tc.tile_pool(name="ps", bufs=1, space="PSUM"))

        _gc = [0]

        def gload(shape, dtype, src, name=None):
            _gc[0] += 1
            t = const.tile(shape, dtype, name=name or f"cst{_gc[0]}")
            nc.gpsimd.dma_start(t, src)
            return t

        w8hh0 = gload([P, KP, 2, 3 * H], f8, d_w8hh0.rearrange("k p j n -> p k j n"))
        w8ih1 = gload([P, KP, 2, 3 * H], f8, d_w8ih1.rearrange("k p j n -> p k j n"))
        w8hh1 = gload([P, KP, 2, 3 * H], f8, d_w8hh1.rearrange("k p j n -> p k j n"))
        w8x = gload([P, 1, 2, 3 * H], f8, d_w8x.rearrange("k p j n -> p k j n"))
        w8fc = gload([P, KP, 2, D], f8, d_w8fc.rearrange("k p j n -> p k j n"))
        const0 = gload([P, 4 * H], bf16, d_const0[:])
        const1 = gload([P, 4 * H], bf16, d_const1[:])
        constfc = gload([P, D], bf16, d_constfc[:])
        ident = gload([P, P], bf16, d_ident[:])
        h0 = gload([P, H], bf16, d_h0[:])
        h1 = gload([P, H], bf16, d_h1[:])
        h0t8a_st = gload([P, 4, P], f8, d_h0t8[:, 0:4, :], name="h0t8a_st")
        h0t8b_st = gload([P, 4, P], f8, d_h0t8[:, 4:8, :], name="h0t8b_st")
        h1t8a_st = gload([P, 4, P], f8, d_h1t8[:, 0:4, :], name="h1t8a_st")
        h1t8b_st = gload([P, 4, P], f8, d_h1t8[:, 4:8, :], name="h1t8b_st")
        xt8_st = gload([P, 2, P], f8, d_xt8[:], name="xt8_st")

        state = {"h0t8": (h0t8a_st, h0t8b_st),
                 "h1t8": (h1t8a_st, h1t8b_st),
                 "xt8": xt8_st}

        def pair_ap(t8pair, kt):
            a, b = t8pair
            return (a if kt < 2 else b)[:, 2 * (kt % 2):2 * (kt % 2) + 2, :]

        # gate -> (const offset, N base in weight cols)
        GOFFS = {"r": (0, 0), "z": (1, H), "hn": (2, 2 * H), "in": (3, 2 * H)}

        def emit_Ihh(psg, v, hh_t8, w_hh, cst, pfx):
            """I-const + hh-pair matmuls for gates r/z/hn of half v."""
            for g in ("r", "z", "hn"):
                pt = ps.tile([P, 512], f32, tag=f"{g}{v}", name=f"ps_{pfx}{g}{v}")
                psg[(g, v)] = pt
                co, _ = GOFFS[g]
                nc.tensor.matmul(
                    pt, ident,
                    cst[:, co * H + v * 512:co * H + (v + 1) * 512],
                    start=True, stop=False)
            for k in range(KP):
                lhsT = pair_ap(hh_t8, k)
                for g in ("r", "z", "hn"):
                    _, nb = GOFFS[g]
                    nc.tensor.matmul(
                        psg[(g, v)], lhsT,
                        w_hh[:, k, :, nb + v * 512:nb + (v + 1) * 512],
                        start=False, stop=(g == "hn" and k == KP - 1),
                        perf_mode=DR)

        def emit_ih(psg, ih_t8, n_pairs, w_ih, cst, pfx, with_in=True):
            """ih-pair matmuls for gates r/z(/in) of both halves; the
            'in' bank gets its I-const here too (it time-shares with
            the FC bank)."""
            for v in (0, 1):
                if with_in:
                    pt = ps.tile([P, 512], f32, tag=f"in{v}", name=f"ps_{pfx}in{v}")
                    psg[("in", v)] = pt
                    co, _ = GOFFS["in"]
                    nc.tensor.matmul(
                        pt, ident,
                        cst[:, co * H + v * 512:co * H + (v + 1) * 512],
                        start=True, stop=False)
                for k in range(n_pairs):
                    lhsT = (ih_t8[:, 0:2, :] if n_pairs == 1
                            else pair_ap(ih_t8, k))
                    for g in ("r", "z", "in"):
                        _, nb = GOFFS[g]
                        nc.tensor.matmul(
                            psg[(g, v)], lhsT,
                            w_ih[:, k, :, nb + v * 512:nb + (v + 1) * 512],
                            start=False, stop=k == n_pairs - 1,
                            perf_mode=DR)

        def cell(lid, psg, h, t8_state, last_of_body):
            """GRU cell: h (bf16, in place) and fp8 transposed pairs."""
            def ctile(shape, dt, tg, bufs=1):
                return act.tile(shape, dt, tag=tg, bufs=bufs, name=f"cl_{tg}")
            r_sb = [ctile([P, 512], bf16, f"r{lid}{v}") for v in (0, 1)]
            z_sb = [ctile([P, 512], bf16, f"z{lid}{v}") for v in (0, 1)]
            wm = [ctile([P, 512], bf16, f"w{lid}{v}") for v in (0, 1)]
            am = [ctile([P, 512], bf16, f"am{lid}{v}") for v in (0, 1)]
            g1 = [ctile([P, 512], f32, f"a{lid}{v}") for v in (0, 1)]
            npre = [ctile([P, 512], f32, f"b{lid}{v}") for v in (0, 1)]
            n_t = [ctile([P, 512], bf16, f"n{lid}{v}") for v in (0, 1)]
            u = [ctile([P, 512], bf16, f"u{lid}{v}") for v in (0, 1)]
            htbf = [ctile([P, 4, P], bf16, f"htbf{lid}{v}", bufs=2)
                    for v in (0, 1)]
            if last_of_body:
                out = t8_state
            else:
                out = (act.tile([P, 4, P], f8, tag=f"t8_{lid}a", bufs=2,
                                name=f"t8_{lid}a"),
                       act.tile([P, 4, P], f8, tag=f"t8_{lid}b", bufs=2,
                                name=f"t8_{lid}b"))
            for v in (0, 1):
                nc.scalar.activation(r_sb[v], psg[("r", v)], AF.Sigmoid,
                                     scale=1.0 / SC)
                nc.scalar.activation(z_sb[v], psg[("z", v)], AF.Sigmoid,
                                     scale=1.0 / SC)
                nc.gpsimd.tensor_scalar(wm[v], z_sb[v], -1.0, 1.0,
                                        ALU.mult, ALU.add)
                nc.gpsimd.tensor_mul(am[v], z_sb[v], h[:, v * 512:(v + 1) * 512])
            for v in (0, 1):
                nc.vector.tensor_mul(g1[v], r_sb[v], psg[("hn", v)])
                nc.vector.tensor_add(npre[v], g1[v], psg[("in", v)])
            for v in (0, 1):
                nc.scalar.activation(n_t[v], npre[v], AF.Tanh, scale=1.0 / SC)
            for v in (0, 1):
                nc.vector.tensor_mul(u[v], wm[v], n_t[v])
                nc.vector.tensor_add(h[:, v * 512:(v + 1) * 512], am[v], u[v])
                nc.sync.dma_start_transpose(htbf[v], h[:, v * 512:(v + 1) * 512])
            for v in (0, 1):
                nc.scalar.mul(out[v], htbf[v], SH)
            return out

        def emit_fc(h1t8_cur, res_row, make_xt8):
            """FC + softmax/sigmoid + res store (+ x->fp8 pair tile).
            Time-shares the in-lo PSUM bank."""
            ps_fc = ps.tile([P, 512], f32, tag="in0", name="ps_fc")
            nc.tensor.matmul(ps_fc[:, :D], ident, constfc, start=True,
                             stop=False)
            for k in range(KP):
                nc.tensor.matmul(ps_fc[:, :D], pair_ap(h1t8_cur, k),
                                 w8fc[:, k, :, :], start=False,
                                 stop=k == KP - 1, perf_mode=DR)
            xf = act.tile([P, D], f32, tag="xf", bufs=2, name="xf")
            sigp = act.tile([P, 47], f32, tag="sigp", bufs=2, name="sigp")
            sign = act.tile([P, 47], f32, tag="sign", bufs=2, name="sign")
            s12 = act.tile([P, 2], f32, tag="s12", bufs=2, name="s12")
            r12 = act.tile([P, 2], f32, tag="r12", bufs=2, name="r12")
            nc.scalar.activation(xf[:, 47:D], ps_fc[:, 47:D], AF.Sigmoid,
                                 scale=1.0 / SC)
            nc.scalar.activation(sigp, ps_fc[:, 0:47], AF.Sigmoid,
                                 scale=1.0 / SC)
            nc.scalar.activation(sign, ps_fc[:, 0:47], AF.Sigmoid,
                                 scale=-1.0 / SC)
            nc.vector.reciprocal(sign, sign)
            nc.vector.scalar_tensor_tensor(
                xf[:, 0:32], sigp[:, 0:32], 1.0, sign[:, 0:32],
                ALU.mult, ALU.mult, accum_out=s12[:, 0:1])
            nc.vector.scalar_tensor_tensor(
                xf[:, 32:47], sigp[:, 32:47], 1.0, sign[:, 32:47],
                ALU.mult, ALU.mult, accum_out=s12[:, 1:2])
            nc.vector.reciprocal(r12, s12)
            nc.vector.tensor_scalar_mul(xf[:, 0:32], xf[:, 0:32], r12[:, 0:1])
            nc.vector.tensor_scalar_mul(xf[:, 32:47], xf[:, 32:47], r12[:, 1:2])
            nc.gpsimd.dma_start(d_res[bass.ds(res_row, P), :], xf)
            if not make_xt8:
                return None
            xbf = act.tile([P, D], bf16, tag="xbf", bufs=2, name="xbf")
            nc.vector.tensor_copy(xbf, xf)
            xtbf = act.tile([P, 2, P], bf16, tag="xtbf", bufs=2, name="xtbf")
            nc.scalar.dma_start_transpose(xtbf, xbf)
            xt8 = act.tile([P, 2, P], f8, tag="xt8", bufs=2, name="xt8")
            nc.scalar.mul(xt8, xtbf, SH)
            return xt8

        def emit_step(fc_row, last_of_body, do_fc):
            cur_h0t8 = state["h0t8"]
            cur_h1t8 = state["h1t8"]

            psg0 = {}
            # A: L0 I+hh lo
            emit_Ihh(psg0, 0, cur_h0t8, w8hh0, const0, "l0")
            # B: FC of the previous step (+ x -> xt8 chain)
            if do_fc:
                state["xt8"] = emit_fc(cur_h1t8, fc_row, make_xt8=True)
            # C: L0 I+hh hi
            emit_Ihh(psg0, 1, cur_h0t8, w8hh0, const0, "l0")
            # D: L0 x-gemm (r/z/in both halves; allocates in-banks)
            emit_ih(psg0, state["xt8"], 1, w8x, const0, "l0")
            # E: L1 I+hh both halves
            psg1 = {}
            emit_Ihh(psg1, 0, cur_h1t8, w8hh1, const1, "l1")
            emit_Ihh(psg1, 1, cur_h1t8, w8hh1, const1, "l1")
            # F: cell 0 (non-PE)
            new_h0t8 = cell(0, psg0, h0, (h0t8a_st, h0t8b_st), last_of_body)
            # G: L1 ih (r/z/in both halves)
            emit_ih(psg1, new_h0t8, KP, w8ih1, const1, "l1")
            # H: cell 1
            new_h1t8 = cell(1, psg1, h1, (h1t8a_st, h1t8b_st), last_of_body)

            state["h0t8"] = new_h0t8
            state["h1t8"] = new_h1t8

        # ---- prologue: step 0 (x from init, no FC) ----
        emit_step(fc_row=0, last_of_body=True, do_fc=False)

        # ---- main loop: iterations cover steps 1..(n_iter*unroll) ----
        et = mybir.EngineType
        with tc.For_i(0, n_iter, 1,
                      hint_engines=(et.PE, et.DVE, et.Activation,
                                    et.SP, et.Pool)) as iv:
            row_base = iv * (unroll * P)
            for j in range(unroll):
                emit_step(fc_row=row_base + j * P,
                          last_of_body=(j == unroll - 1), do_fc=True)

        # ---- tail: steps (n_iter*unroll + 1) .. (t_steps - 1) ----
        t0 = n_iter * unroll    # first tail fc row
        for m in range(unroll - 1):
            emit_step(fc_row=(t0 + m) * P, last_of_body=False, do_fc=True)
        # final FC for the last step
        emit_fc(state["h1t8"], (t_steps - 1) * P, make_xt8=False)

    _dedupe_ldweights(nc, mybir)
    nc.finalize()
    return nc


def _dedupe_ldweights(nc, mybir):
    """Drop redundant back-to-back Ldweights of the same stationary tile."""
    import orjson
    removed = 0
    for func in nc.m.functions:
        for blk in func.blocks:
            last_key = None
            kept = []
            blk_removed = 0
            for inst in blk.instructions:
                if getattr(inst, "engine", None) == mybir.EngineType.PE:
                    d = orjson.loads(mybir.instruction_to_pretty_json_string(inst))
                    op = d.get("opcode")
                    if op == "Ldweights":
                        si = d.get("sync_info") or {}
                        key = orjson.dumps(
                            (d.get("ins"), d.get("tile_position"),
                             d.get("tile_size"), d.get("perf_mode"),
                             d.get("is_transpose")))
                        if (key == last_key and not si.get("on_wait")
                                and not si.get("on_update")):
                            removed += 1
                            blk_removed += 1
                            continue
                        last_key = key
                kept.append(inst)
            if blk_removed:
                blk.instructions[:] = kept
    return removed


def _host_prep(inputs):
    """Build per-core input maps."""
    bf = ml_dtypes.bfloat16
    e4 = ml_dtypes.float8_e4m3
    embed = np.ascontiguousarray(np.asarray(inputs["embed"], dtype=np.float32))
    dynamics = np.asarray(inputs["dynamics"], dtype=np.float32)
    W_ih0 = np.asarray(inputs["W_ih0"], dtype=np.float32)
    W_hh0 = np.asarray(inputs["W_hh0"], dtype=np.float32)
    b_ih0 = np.asarray(inputs["b_ih0"], dtype=np.float32)
    b_hh0 = np.asarray(inputs["b_hh0"], dtype=np.float32)
    W_ih1 = np.asarray(inputs["W_ih1"], dtype=np.float32)
    W_hh1 = np.asarray(inputs["W_hh1"], dtype=np.float32)
    b_ih1 = np.asarray(inputs["b_ih1"], dtype=np.float32)
    b_hh1 = np.asarray(inputs["b_hh1"], dtype=np.float32)
    W_fc = np.asarray(inputs["W_fc"], dtype=np.float32)
    b_fc = np.asarray(inputs["b_fc"], dtype=np.float32)

    glob = embed[:, :H]
    h0i = embed[:, H:2 * H]
    h1i = embed[:, 2 * H:3 * H]
    x0 = dynamics[:, 0, :]

    c0 = (glob.astype(np.float64) @ W_ih0[:, :H].T.astype(np.float64)).astype(np.float32)
    c0 += b_ih0

    def pairize(wT, kp):
        K, N = wT.shape
        assert K == kp * 2 * P
        w8 = np.asarray(wT * SW, dtype=e4)
        return np.ascontiguousarray(w8.reshape(kp, 2, P, N).transpose(0, 2, 1, 3))

    def bcast(row):
        return np.broadcast_to(row, (P, row.shape[0]))

    const1 = np.concatenate([
        bcast((b_ih1 + b_hh1)[:H]),
        bcast((b_ih1 + b_hh1)[H:2 * H]),
        bcast(b_hh1[2 * H:]),
        bcast(b_ih1[2 * H:]),
    ], axis=1) * SC

    shared = {
        "w8hh0": pairize(np.ascontiguousarray(W_hh0.T), KP),
        "w8ih1": pairize(np.ascontiguousarray(W_ih1.T), KP),
        "w8hh1": pairize(np.ascontiguousarray(W_hh1.T), KP),
        "w8x": pairize(np.ascontiguousarray(W_ih0[:, H:].T), 1),
        "w8fc": pairize(np.ascontiguousarray(W_fc.T), KP),
        "const1": np.ascontiguousarray(const1).astype(bf),
        "constfc": np.ascontiguousarray(bcast(b_fc) * SC).astype(bf),
        "ident": np.eye(P, dtype=np.float32).astype(bf),
    }

    def t8(hslice, nslots):
        hT = np.ascontiguousarray(np.asarray(hslice, np.float32).T)
        return np.ascontiguousarray(
            np.asarray(hT * SH, dtype=e4).reshape(nslots, P, P).transpose(1, 0, 2))

    in_maps = []
    for c in range(NCORES):
        s = slice(c * P, (c + 1) * P)
        m = dict(shared)
        h0bf = h0i[s].astype(bf)
        h1bf = h1i[s].astype(bf)
        m["h0_init"] = np.ascontiguousarray(h0bf)
        m["h1_init"] = np.ascontiguousarray(h1bf)
        m["h0t8_init"] = t8(h0bf.astype(np.float32), 8)
        m["h1t8_init"] = t8(h1bf.astype(np.float32), 8)
        m["xt8_init"] = t8(x0[s], 2)
        const0 = np.concatenate([
            c0[s, :H] + b_hh0[:H],
            c0[s, H:2 * H] + b_hh0[H:2 * H],
            np.broadcast_to(b_hh0[2 * H:], (P, H)),
            c0[s, 2 * H:],
        ], axis=1) * SC
        m["const0"] = np.ascontiguousarray(const0).astype(bf)
        in_maps.append(m)
    return in_maps


def _install_neff_cache():
    """Cache walrus-compiled NEFFs keyed by BIR hash."""
    import hashlib
    import shutil
    import concourse.bass_utils as bu
    import concourse.bass2jax as b2j

    if getattr(bu, "_decoder_neff_cache", False):
        return
    orig = bu.compile_bir_kernel

    def cached(bir_json, tmpdir, neff_name="file.neff"):
        try:
            h = hashlib.sha256(bir_json).hexdigest()[:32]
            cdir = os.path.join(os.path.expanduser("~"), ".cache", "bass_neff")
            os.makedirs(cdir, exist_ok=True)
            cpath = os.path.join(cdir, h + ".neff")
            if os.path.exists(cpath):
                dst = os.path.join(tmpdir, "sg00")
                os.makedirs(dst, exist_ok=True)
                out = os.path.join(dst, neff_name)
                shutil.copy(cpath, out)
                return out
            out = orig(bir_json, tmpdir, neff_name)
            shutil.copy(out, cpath)
            return out
        except Exception:
            return orig(bir_json, tmpdir, neff_name)

    bu.compile_bir_kernel = cached
    b2j.compile_bir_kernel = cached
    bu._decoder_neff_cache = True


def _get_nc():
    key = ("v4", T, UNROLL)
    if key not in _prog_cache:
        _prog_cache[key] = _build_program(T, unroll=UNROLL)
    return _prog_cache[key]


def kernel(**inputs):
    from concourse.bass_utils import run_bass_kernel_spmd

    _install_neff_cache()
    nc = _get_nc()
    in_maps = _host_prep(inputs)
    out = run_bass_kernel_spmd(nc, in_maps, core_ids=list(range(NCORES)))
    res = np.concatenate(
        [r["res"].reshape(T, P, D).transpose(1, 0, 2) for r in out.results],
        axis=0)
    return np.ascontiguousarray(res, dtype=np.float32)


# revision 14
# speedup vs baseline: 2.8430x; 1.1597x over previous
"""Trainium2 Bass kernel for nn_Decoder_78305843741218.

2-layer GRU decoder, autoregressive over T=256 steps, batch 1024.
Sharding: data-parallel over batch -> 128 samples/core on 8 cores.

v4 design (per core, per step):
  - All big GEMMs in fp8-e4m3 DoubleRow (2 contraction rows/cycle):
    stationary = transposed activations as fp8 pair-tiles [128,2,128]
    (x16), moving = W^T fp8 pair-chunks [128,2,512] (x64).  PSUM holds
    1024x the true scale; descaled free via activation-scale (1/1024).
  - All weights SBUF-resident in fp8 (~10.5 MB) - no HBM streaming.
  - Per-feature constants (glob @ W_ih0[:,:H].T + biases, x1024, bf16)
    enter PSUM via identity-stationary matmuls.
  - Gates processed in 512-wide halves; 8 PSUM banks = {r,z,hn}x{lo,hi}
    + in-lo + in-hi; the FC time-shares the in-lo bank.
  - Software pipeline: the FC + softmax + x->fp8 chain of step t-1 is
    emitted inside step t's PE stream, covered by L0's I+hh gemms, so
    the PE never waits for the autoregressive x loop.
    Program = prologue (step 0, no FC) + For_i(31) x 8 macro-steps +
    7 tail macro-steps + final FC.
  - Recurrent h0/h1 carried in bf16 (validated: rel err unchanged);
    the cell update is h' = z*h + (1-z)*n with z*h and (1-z) computed
    on GpSimd off the critical path; r*ghn / +gin on DVE from PSUM;
    Tanh/Sigmoid on ScalarE (one ACT table set).
  - h^T via one batched DMA transpose per 512-half (SP hwdge queue;
    x's on the ACT queue), then one ScalarE x16 copy to fp8 pairs.
  - Redundant Ldweights deduped post-Tile.

Output is stored T-major (T*128, D) per core; host reassembles.
"""

import os
import numpy as np
import ml_dtypes

H = 1024
D = 256
T = 256
P = 128
NCORES = 8
KP = 4            # fp8 pair k-tiles for H-dim contraction
SW = 64.0         # weight scale in fp8
SH = 16.0         # activation scale in fp8
SC = SW * SH      # psum scale
UNROLL = 32

_prog_cache = {}


def _build_program(t_steps=T, unroll=UNROLL):
    import concourse.bass as bass
    import concourse.bacc as bacc
    import concourse.mybir as mybir
    import concourse.tile as tile
    from contextlib import ExitStack

    f32 = mybir.dt.float32
    bf16 = mybir.dt.bfloat16
    f8 = mybir.dt.float8e4
    AF = mybir.ActivationFunctionType
    ALU = mybir.AluOpType
    DR = mybir.MatmulPerfMode.DoubleRow

    assert t_steps % unroll == 0
    n_iter = t_steps // unroll - 1   # one iteration's worth is peeled

    nc = bacc.Bacc(None, target_bir_lowering=False)

    # ---- I/O ----
    d_w8hh0 = nc.dram_tensor("w8hh0", (KP, P, 2, 3 * H), f8, kind="ExternalInput")
    d_w8ih1 = nc.dram_tensor("w8ih1", (KP, P, 2, 3 * H), f8, kind="ExternalInput")
    d_w8hh1 = nc.dram_tensor("w8hh1", (KP, P, 2, 3 * H), f8, kind="ExternalInput")
    d_w8x = nc.dram_tensor("w8x", (1, P, 2, 3 * H), f8, kind="ExternalInput")
    d_w8fc = nc.dram_tensor("w8fc", (KP, P, 2, D), f8, kind="ExternalInput")
    d_const0 = nc.dram_tensor("const0", (P, 4 * H), bf16, kind="ExternalInput")
    d_const1 = nc.dram_tensor("const1", (P, 4 * H), bf16, kind="ExternalInput")
    d_constfc = nc.dram_tensor("constfc", (P, D), bf16, kind="ExternalInput")
    d_ident = nc.dram_tensor("ident", (P, P), bf16, kind="ExternalInput")
    d_h0 = nc.dram_tensor("h0_init", (P, H), bf16, kind="ExternalInput")
    d_h1 = nc.dram_tensor("h1_init", (P, H), bf16, kind="ExternalInput")
    d_h0t8 = nc.dram_tensor("h0t8_init", (P, 8, P), f8, kind="ExternalInput")
    d_h1t8 = nc.dram_tensor("h1t8_init", (P, 8, P), f8, kind="ExternalInput")
    d_xt8 = nc.dram_tensor("xt8_init", (P, 2, P), f8, kind="ExternalInput")
    d_res = nc.dram_tensor("res", (t_steps * P, D), f32, kind="ExternalOutput")

    with tile.TileContext(nc) as tc, ExitStack() as ctx:
        const = ctx.enter_context(tc.tile_pool(name="const", bufs=1))
        act = ctx.enter_context(tc.tile_pool(name="act", bufs=2))
        ps = ctx.enter_context(tc.tile_pool(name="ps", bufs=1, space="PSUM"))

        _gc = [0]

        def gload(shape, dtype, src, name=None):
            _gc[0] += 1
            t = const.tile(shape, dtype, name=name or f"cst{_gc[0]}")
            nc.gpsimd.dma_start(t, src)
            return t

        w8hh0 = gload([P, KP, 2, 3 * H], f8, d_w8hh0.rearrange("k p j n -> p k j n"))
        w8ih1 = gload([P, KP, 2, 3 * H], f8, d_w8ih1.rearrange("k p j n -> p k j n"))
        w8hh1 = gload([P, KP, 2, 3 * H], f8, d_w8hh1.rearrange("k p j n -> p k j n"))
        w8x = gload([P, 1, 2, 3 * H], f8, d_w8x.rearrange("k p j n -> p k j n"))
        w8fc = gload([P, KP, 2, D], f8, d_w8fc.rearrange("k p j n -> p k j n"))
        const0 = gload([P, 4 * H], bf16, d_const0[:])
        const1 = gload([P, 4 * H], bf16, d_const1[:])
        constfc = gload([P, D], bf16, d_constfc[:])
        ident = gload([P, P], bf16, d_ident[:])
        h0 = gload([P, H], bf16, d_h0[:])
        h1 = gload([P, H], bf16, d_h1[:])
        h0t8a_st = gload([P, 4, P], f8, d_h0t8[:, 0:4, :], name="h0t8a_st")
        h0t8b_st = gload([P, 4, P], f8, d_h0t8[:, 4:8, :], name="h0t8b_st")
        h1t8a_st = gload([P, 4, P], f8, d_h1t8[:, 0:4, :], name="h1t8a_st")
        h1t8b_st = gload([P, 4, P], f8, d_h1t8[:, 4:8, :], name="h1t8b_st")
        xt8_st = gload([P, 2, P], f8, d_xt8[:], name="xt8_st")

        state = {"h0t8": (h0t8a_st, h0t8b_st),
                 "h1t8": (h1t8a_st, h1t8b_st),
                 "xt8": xt8_st}

        def pair_ap(t8pair, kt):
            a, b = t8pair
            return (a if kt < 2 else b)[:, 2 * (kt % 2):2 * (kt % 2) + 2, :]

        # gate -> (const offset, N base in weight cols)
        GOFFS = {"r": (0, 0), "z": (1, H), "hn": (2, 2 * H), "in": (3, 2 * H)}

        def emit_Ihh(psg, v, hh_t8, w_hh, cst, pfx):
            """I-const + hh-pair matmuls for gates r/z/hn of half v."""
            for g in ("r", "z", "hn"):
                pt = ps.tile([P, 512], f32, tag=f"{g}{v}", name=f"ps_{pfx}{g}{v}")
                psg[(g, v)] = pt
                co, _ = GOFFS[g]
                nc.tensor.matmul(
                    pt, ident,
                    cst[:, co * H + v * 512:co * H + (v + 1) * 512],
                    start=True, stop=False)
            for k in range(KP):
                lhsT = pair_ap(hh_t8, k)
                for g in ("r", "z", "hn"):
                    _, nb = GOFFS[g]
                    nc.tensor.matmul(
                        psg[(g, v)], lhsT,
                        w_hh[:, k, :, nb + v * 512:nb + (v + 1) * 512],
                        start=False, stop=(g == "hn" and k == KP - 1),
                        perf_mode=DR)

        def emit_ih(psg, ih_t8, n_pairs, w_ih, cst, pfx, with_in=True):
            """ih-pair matmuls for gates r/z(/in) of both halves; the
            'in' bank gets its I-const here too (it time-shares with
            the FC bank)."""
            for v in (0, 1):
                if with_in:
                    pt = ps.tile([P, 512], f32, tag=f"in{v}", name=f"ps_{pfx}in{v}")
                    psg[("in", v)] = pt
                    co, _ = GOFFS["in"]
                    nc.tensor.matmul(
                        pt, ident,
                        cst[:, co * H + v * 512:co * H + (v + 1) * 512],
                        start=True, stop=False)
                for k in range(n_pairs):
                    lhsT = (ih_t8[:, 0:2, :] if n_pairs == 1
                            else pair_ap(ih_t8, k))
                    for g in ("r", "z", "in"):
                        _, nb = GOFFS[g]
                        nc.tensor.matmul(
                            psg[(g, v)], lhsT,
                            w_ih[:, k, :, nb + v * 512:nb + (v + 1) * 512],
                            start=False, stop=k == n_pairs - 1,
                            perf_mode=DR)

        def cell(lid, psg, h, t8_state, last_of_body):
            """GRU cell: h (bf16, in place) and fp8 transposed pairs."""
            def ctile(shape, dt, tg, bufs=1):
                return act.tile(shape, dt, tag=tg, bufs=bufs, name=f"cl_{tg}")
            r_sb = [ctile([P, 512], bf16, f"r{lid}{v}") for v in (0, 1)]
            z_sb = [ctile([P, 512], bf16, f"z{lid}{v}") for v in (0, 1)]
            wm = [ctile([P, 512], bf16, f"w{lid}{v}") for v in (0, 1)]
            am = [ctile([P, 512], bf16, f"am{lid}{v}") for v in (0, 1)]
            g1 = [ctile([P, 512], f32, f"a{lid}{v}") for v in (0, 1)]
            npre = [ctile([P, 512], f32, f"b{lid}{v}") for v in (0, 1)]
            n_t = [ctile([P, 512], bf16, f"n{lid}{v}") for v in (0, 1)]
            u = [ctile([P, 512], bf16, f"u{lid}{v}") for v in (0, 1)]
            htbf = [ctile([P, 4, P], bf16, f"htbf{lid}{v}", bufs=2)
                    for v in (0, 1)]
            if last_of_body:
                out = t8_state
            else:
                out = (act.tile([P, 4, P], f8, tag=f"t8_{lid}a", bufs=2,
                                name=f"t8_{lid}a"),
                       act.tile([P, 4, P], f8, tag=f"t8_{lid}b", bufs=2,
                                name=f"t8_{lid}b"))
            for v in (0, 1):
                nc.scalar.activation(r_sb[v], psg[("r", v)], AF.Sigmoid,
                                     scale=1.0 / SC)
                nc.scalar.activation(z_sb[v], psg[("z", v)], AF.Sigmoid,
                                     scale=1.0 / SC)
                nc.gpsimd.tensor_scalar(wm[v], z_sb[v], -1.0, 1.0,
                                        ALU.mult, ALU.add)
                nc.gpsimd.tensor_mul(am[v], z_sb[v], h[:, v * 512:(v + 1) * 512])
            for v in (0, 1):
                nc.vector.tensor_mul(g1[v], r_sb[v], psg[("hn", v)])
                nc.vector.tensor_add(npre[v], g1[v], psg[("in", v)])
            for v in (0, 1):
                nc.scalar.activation(n_t[v], npre[v], AF.Tanh, scale=1.0 / SC)
            for v in (0, 1):
                nc.vector.tensor_mul(u[v], wm[v], n_t[v])
                nc.vector.tensor_add(h[:, v * 512:(v + 1) * 512], am[v], u[v])
                nc.sync.dma_start_transpose(htbf[v], h[:, v * 512:(v + 1) * 512])
            for v in (0, 1):
                nc.scalar.mul(out[v], htbf[v], SH)
            return out

        def emit_fc(h1t8_cur, res_row, make_xt8):
            """FC + softmax/sigmoid + res store (+ x->fp8 pair tile).
            Time-shares the in-lo PSUM bank."""
            ps_fc = ps.tile([P, 512], f32, tag="in0", name="ps_fc")
            nc.tensor.matmul(ps_fc[:, :D], ident, constfc, start=True,
                             stop=False)
            for k in range(KP):
                nc.tensor.matmul(ps_fc[:, :D], pair_ap(h1t8_cur, k),
                                 w8fc[:, k, :, :], start=False,
                                 stop=k == KP - 1, perf_mode=DR)
            xf = act.tile([P, D], f32, tag="xf", bufs=2, name="xf")
            sigp = act.tile([P, 47], f32, tag="sigp", bufs=2, name="sigp")
            sign = act.tile([P, 47], f32, tag="sign", bufs=2, name="sign")
            s12 = act.tile([P, 2], f32, tag="s12", bufs=2, name="s12")
            r12 = act.tile([P, 2], f32, tag="r12", bufs=2, name="r12")
            nc.scalar.activation(xf[:, 47:D], ps_fc[:, 47:D], AF.Sigmoid,
                                 scale=1.0 / SC)
            nc.scalar.activation(sigp, ps_fc[:, 0:47], AF.Sigmoid,
                                 scale=1.0 / SC)
            nc.scalar.activation(sign, ps_fc[:, 0:47], AF.Sigmoid,
                                 scale=-1.0 / SC)
            nc.vector.reciprocal(sign, sign)
            nc.vector.scalar_tensor_tensor(
                xf[:, 0:32], sigp[:, 0:32], 1.0, sign[:, 0:32],
                ALU.mult, ALU.mult, accum_out=s12[:, 0:1])
            nc.vector.scalar_tensor_tensor(
                xf[:, 32:47], sigp[:, 32:47], 1.0, sign[:, 32:47],
                ALU.mult, ALU.mult, accum_out=s12[:, 1:2])
            nc.vector.reciprocal(r12, s12)
            nc.vector.tensor_scalar_mul(xf[:, 0:32], xf[:, 0:32], r12[:, 0:1])
            nc.vector.tensor_scalar_mul(xf[:, 32:47], xf[:, 32:47], r12[:, 1:2])
            nc.gpsimd.dma_start(d_res[bass.ds(res_row, P), :], xf)
            if not make_xt8:
                return None
            xbf = act.tile([P, D], bf16, tag="xbf", bufs=2, name="xbf")
            nc.vector.tensor_copy(xbf, xf)
            return xbf

        def emit_xtr(xbf):
            """x^T on the PE (2 transpose matmuls into the spare half of
            the in1 bank) + one ScalarE x16 cast to fp8 pairs."""
            ps_xt = ps.tile([P, 2, P], bf16, tag="in1", name="ps_xt")
            for j in range(2):
                nc.tensor.transpose(ps_xt[:, j, :], xbf[:, j * P:(j + 1) * P],
                                    ident)
            xt8 = act.tile([P, 2, P], f8, tag="xt8", bufs=2, name="xt8")
            nc.scalar.mul(xt8, ps_xt, SH)
            return xt8

        def emit_step(fc_row, last_of_body, do_fc):
            cur_h0t8 = state["h0t8"]
            cur_h1t8 = state["h1t8"]

            psg0 = {}
            # A: L0 I+hh lo
            emit_Ihh(psg0, 0, cur_h0t8, w8hh0, const0, "l0")
            # B: FC of the previous step (acts/softmax on ACT/DVE)
            xbf = emit_fc(cur_h1t8, fc_row, make_xt8=True) if do_fc else None
            # C: L0 I+hh hi
            emit_Ihh(psg0, 1, cur_h0t8, w8hh0, const0, "l0")
            # B2: x^T on the PE now that xbf has had time to land
            if xbf is not None:
                state["xt8"] = emit_xtr(xbf)
            # D: L0 x-gemm (r/z/in both halves; allocates in-banks)
            emit_ih(psg0, state["xt8"], 1, w8x, const0, "l0")
            # E: L1 I+hh both halves
            psg1 = {}
            emit_Ihh(psg1, 0, cur_h1t8, w8hh1, const1, "l1")
            emit_Ihh(psg1, 1, cur_h1t8, w8hh1, const1, "l1")
            # F: cell 0 (non-PE)
            new_h0t8 = cell(0, psg0, h0, (h0t8a_st, h0t8b_st), last_of_body)
            # G: L1 ih (r/z/in both halves)
            emit_ih(psg1, new_h0t8, KP, w8ih1, const1, "l1")
            # H: cell 1
            new_h1t8 = cell(1, psg1, h1, (h1t8a_st, h1t8b_st), last_of_body)

            state["h0t8"] = new_h0t8
            state["h1t8"] = new_h1t8

        # ---- prologue: step 0 (x from init, no FC) ----
        emit_step(fc_row=0, last_of_body=True, do_fc=False)

        # ---- main loop: iterations cover steps 1..(n_iter*unroll) ----
        et = mybir.EngineType
        with tc.For_i(0, n_iter, 1,
                      hint_engines=(et.PE, et.DVE, et.Activation,
                                    et.SP, et.Pool)) as iv:
            row_base = iv * (unroll * P)
            for j in range(unroll):
                emit_step(fc_row=row_base + j * P,
                          last_of_body=(j == unroll - 1), do_fc=True)

        # ---- tail: steps (n_iter*unroll + 1) .. (t_steps - 1) ----
        t0 = n_iter * unroll    # first tail fc row
        for m in range(unroll - 1):
            emit_step(fc_row=(t0 + m) * P, last_of_body=False, do_fc=True)
        # final FC for the last step
        emit_fc(state["h1t8"], (t_steps - 1) * P, make_xt8=False)

    _dedupe_ldweights(nc, mybir)
    nc.finalize()
    return nc


def _dedupe_ldweights(nc, mybir):
    """Drop redundant back-to-back Ldweights of the same stationary tile."""
    import orjson
    removed = 0
    for func in nc.m.functions:
        for blk in func.blocks:
            last_key = None
            kept = []
            blk_removed = 0
            for inst in blk.instructions:
                if getattr(inst, "engine", None) == mybir.EngineType.PE:
                    d = orjson.loads(mybir.instruction_to_pretty_json_string(inst))
                    op = d.get("opcode")
                    if op == "Ldweights":
                        si = d.get("sync_info") or {}
                        key = orjson.dumps(
                            (d.get("ins"), d.get("tile_position"),
                             d.get("tile_size"), d.get("perf_mode"),
                             d.get("is_transpose")))
                        if (key == last_key and not si.get("on_wait")
                                and not si.get("on_update")):
                            removed += 1
                            blk_removed += 1
                            continue
                        last_key = key
                kept.append(inst)
            if blk_removed:
                blk.instructions[:] = kept
    return removed


def _host_prep(inputs):
    """Build per-core input maps."""
    bf = ml_dtypes.bfloat16
    e4 = ml_dtypes.float8_e4m3
    embed = np.ascontiguousarray(np.asarray(inputs["embed"], dtype=np.float32))
    dynamics = np.asarray(inputs["dynamics"], dtype=np.float32)
    W_ih0 = np.asarray(inputs["W_ih0"], dtype=np.float32)
    W_hh0 = np.asarray(inputs["W_hh0"], dtype=np.float32)
    b_ih0 = np.asarray(inputs["b_ih0"], dtype=np.float32)
    b_hh0 = np.asarray(inputs["b_hh0"], dtype=np.float32)
    W_ih1 = np.asarray(inputs["W_ih1"], dtype=np.float32)
    W_hh1 = np.asarray(inputs["W_hh1"], dtype=np.float32)
    b_ih1 = np.asarray(inputs["b_ih1"], dtype=np.float32)
    b_hh1 = np.asarray(inputs["b_hh1"], dtype=np.float32)
    W_fc = np.asarray(inputs["W_fc"], dtype=np.float32)
    b_fc = np.asarray(inputs["b_fc"], dtype=np.float32)

    glob = embed[:, :H]
    h0i = embed[:, H:2 * H]
    h1i = embed[:, 2 * H:3 * H]
    x0 = dynamics[:, 0, :]

    c0 = (glob.astype(np.float64) @ W_ih0[:, :H].T.astype(np.float64)).astype(np.float32)
    c0 += b_ih0

    def pairize(wT, kp):
        K, N = wT.shape
        assert K == kp * 2 * P
        w8 = np.asarray(wT * SW, dtype=e4)
        return np.ascontiguousarray(w8.reshape(kp, 2, P, N).transpose(0, 2, 1, 3))

    def bcast(row):
        return np.broadcast_to(row, (P, row.shape[0]))

    const1 = np.concatenate([
        bcast((b_ih1 + b_hh1)[:H]),
        bcast((b_ih1 + b_hh1)[H:2 * H]),
        bcast(b_hh1[2 * H:]),
        bcast(b_ih1[2 * H:]),
    ], axis=1) * SC

    shared = {
        "w8hh0": pairize(np.ascontiguousarray(W_hh0.T), KP),
        "w8ih1": pairize(np.ascontiguousarray(W_ih1.T), KP),
        "w8hh1": pairize(np.ascontiguousarray(W_hh1.T), KP),
        "w8x": pairize(np.ascontiguousarray(W_ih0[:, H:].T), 1),
        "w8fc": pairize(np.ascontiguousarray(W_fc.T), KP),
        "const1": np.ascontiguousarray(const1).astype(bf),
        "constfc": np.ascontiguousarray(bcast(b_fc) * SC).astype(bf),
        "ident": np.eye(P, dtype=np.float32).astype(bf),
    }

    def t8(hslice, nslots):
        hT = np.ascontiguousarray(np.asarray(hslice, np.float32).T)
        return np.ascontiguousarray(
            np.asarray(hT * SH, dtype=e4).reshape(nslots, P, P).transpose(1, 0, 2))

    in_maps = []
    for c in range(NCORES):
        s = slice(c * P, (c + 1) * P)
        m = dict(shared)
        h0bf = h0i[s].astype(bf)
        h1bf = h1i[s].astype(bf)
        m["h0_init"] = np.ascontiguousarray(h0bf)
        m["h1_init"] = np.ascontiguousarray(h1bf)
        m["h0t8_init"] = t8(h0bf.astype(np.float32), 8)
        m["h1t8_init"] = t8(h1bf.astype(np.float32), 8)
        m["xt8_init"] = t8(x0[s], 2)
        const0 = np.concatenate([
            c0[s, :H] + b_hh0[:H],
            c0[s, H:2 * H] + b_hh0[H:2 * H],
            np.broadcast_to(b_hh0[2 * H:], (P, H)),
            c0[s, 2 * H:],
        ], axis=1) * SC
        m["const0"] = np.ascontiguousarray(const0).astype(bf)
        in_maps.append(m)
    return in_maps


def _install_neff_cache():
    """Cache walrus-compiled NEFFs keyed by BIR hash."""
    import hashlib
    import shutil
    import concourse.bass_utils as bu
    import concourse.bass2jax as b2j

    if getattr(bu, "_decoder_neff_cache", False):
        return
    orig = bu.compile_bir_kernel

    def cached(bir_json, tmpdir, neff_name="file.neff"):
        try:
            h = hashlib.sha256(bir_json).hexdigest()[:32]
            cdir = os.path.join(os.path.expanduser("~"), ".cache", "bass_neff")
            os.makedirs(cdir, exist_ok=True)
            cpath = os.path.join(cdir, h + ".neff")
            if os.path.exists(cpath):
                dst = os.path.join(tmpdir, "sg00")
                os.makedirs(dst, exist_ok=True)
                out = os.path.join(dst, neff_name)
                shutil.copy(cpath, out)
                return out
            out = orig(bir_json, tmpdir, neff_name)
            shutil.copy(out, cpath)
            return out
        except Exception:
            return orig(bir_json, tmpdir, neff_name)

    bu.compile_bir_kernel = cached
    b2j.compile_bir_kernel = cached
    bu._decoder_neff_cache = True


def _get_nc():
    key = ("v4", T, UNROLL)
    if key not in _prog_cache:
        _prog_cache[key] = _build_program(T, unroll=UNROLL)
    return _prog_cache[key]


def kernel(**inputs):
    from concourse.bass_utils import run_bass_kernel_spmd

    _install_neff_cache()
    nc = _get_nc()
    in_maps = _host_prep(inputs)
    out = run_bass_kernel_spmd(nc, in_maps, core_ids=list(range(NCORES)))
    res = np.concatenate(
        [r["res"].reshape(T, P, D).transpose(1, 0, 2) for r in out.results],
        axis=0)
    return np.ascontiguousarray(res, dtype=np.float32)
